# revision 12
# baseline (speedup 1.0000x reference)
"""Trainium2 Bass kernel for DPBlockVFAStandard (3D local cross-attention
displacement field).

Computation (B=1, C=32, E=16, H=W=D=64):
  fixed_emb  = conv3d(feat_fixed, w, b, pad=1)                    [E,64,64,64]
  moving_emb = conv3d(edge_pad(feat_moving,1), w, b, pad=1)       [E,66,66,66]
  scores[p](h,w,d) = <fixed_emb(h,w,d), moving_emb(h+i,w+j,d+k)>/4, p=(i,j,k)
  attn = softmax_p(scores);  disp_r = sum_p attn_p * R[p,r]       [3,64,64,64]

Sharding: H split into 8 slabs of 8 rows, one per NeuronCore; halo handled
host-side by overlapping input slabs (no collectives).

Per-core device pipeline (all matmuls fp16 -> 1 cycle/row on the PE):
  - moving conv: 9 accumulating fp16 matmuls per PSUM tile, contraction (i,c)
    zero-padded to K=128, 4-way col-tiled over output chunks; per-chunk PSUM
    slices evacuated directly (Act/DVE copies, fp32->fp16) into the final
    plane-stack layout [16h+e, (w,d)] -- no staging DMAs
  - moving planes staged as 3 partition-shifted stacks (fp16 SBUF->SBUF DMA)
  - fixed conv: same -> Q stack [16h+e, w*64+d] fp16
  - scores: 27 elementwise fp16 muls (DVE 2x / Pool) + block-diag fp16 reduce
    matmuls packing 16 offsets per PSUM tile [ (a,w,h) -> 32a+8w+h ]
  - exp on ScalarE (bias -4) -> fp16; R-codebook reduction as 1 fp16 matmul
    per (G, chunk) -> [4r x 8h, 512] PSUM
  - num/den evac to fp16 (Act), reciprocal + muls on DVE in fp16 2x mode
  - fp16 output DMA; host upcasts to fp32
"""

import os

import numpy as np

EMBED = 16
C = 32
H = 64
NCORES = 8
ROWS = H // NCORES          # 8 output rows per core
TEMP = 4.0
EXP_BIAS = -4.0

GP_EVERY = int(os.environ.get("GP_EVERY", "4"))   # every Nth score mul -> Pool
KREP = int(os.environ.get("KREP", "1"))           # body repetitions (timing probe)

_PROG_CACHE = {}


def _radial():
    c = np.array([-1.0, 0.0, 1.0], np.float32)
    R = np.zeros((27, 3), np.float32)
    for p in range(27):
        i, j, k = p // 9, (p // 3) % 3, p % 3
        R[p] = (c[i], c[j], c[k])
    return R


def _host_consts(conv_w, conv_b):
    """Build packed lhsT constant matrices (fp16).

    cw  [128, 704] fp16:
      0:288    fixed conv lhsT, 9 offsets x [128, 32]: cols 0:16 parity-0 row
               (K blocks 0..2 = planes h0..h0+2), cols 16:32 parity-1 row
               (K blocks 1..3)
      288:576  moving conv lhsT (w/TEMP), same layout
      576:704  LRED: 4 variants w, block [128,32], col (8w+h) sums over e of
               partition (16h+e)
    cr  [128, 256] fp16: blocks G in (0,1) of [128, 128];
      row (32a+8w+h) of block G, col (32r+h') = ind(h==h') * wr(p),
      p = G*16+4w+a, wr = R[p,r] for r<3 else 1 (r=3 -> denominator).
      (r at 32-partition stride so psum consumer slices are 32-aligned)
    """
    w = conv_w.astype(np.float32)          # [E, C, 3, 3, 3]
    wm = w / TEMP
    cw = np.zeros((128, 704), np.float32)
    for jk in range(9):
        j, k = jk // 3, jk % 3
        for i in range(3):
            blk = w[:, :, i, j, k].T       # [C, E]
            blkm = wm[:, :, i, j, k].T
            cw[32 * i:32 * i + 32, jk * 32:jk * 32 + 16] = blk
            cw[32 * (i + 1):32 * (i + 1) + 32, jk * 32 + 16:jk * 32 + 32] = blk
            cw[32 * i:32 * i + 32, 288 + jk * 32:288 + jk * 32 + 16] = blkm
            cw[32 * (i + 1):32 * (i + 1) + 32, 288 + jk * 32 + 16:288 + jk * 32 + 32] = blkm
    for wv in range(4):
        base = 576 + 32 * wv
        for h in range(8):
            cw[16 * h:16 * h + 16, base + 8 * wv + h] = 1.0

    R = _radial()
    cr = np.zeros((128, 256), np.float32)
    for G in range(2):
        npg = 16 if G == 0 else 11
        for idx in range(npg):
            p = G * 16 + idx
            a, wv = idx % 4, idx // 4
            for r in range(4):
                val = R[p, r] if r < 3 else 1.0
                for h in range(8):
                    cr[32 * a + 8 * wv + h, G * 128 + 32 * r + h] = val
    return cw.astype(np.float16), cr.astype(np.float16)


def _trace_program():
    import concourse.bacc as bacc
    import concourse.tile as tile
    import concourse.mybir as mybir
    from contextlib import ExitStack

    f32 = mybir.dt.float32
    fp16 = mybir.dt.float16
    Exp = mybir.ActivationFunctionType.Exp

    nc = bacc.Bacc("TRN2", target_bir_lowering=False, debug=False,
                   enable_asserts=True, num_devices=NCORES)
    xfix = nc.dram_tensor("xfix", [10, C, 66 * 66], fp16, kind="ExternalInput")
    xmov = nc.dram_tensor("xmov", [12, C, 68 * 68], fp16, kind="ExternalInput")
    cw_t = nc.dram_tensor("cw", [128, 704], fp16, kind="ExternalInput")
    cr_t = nc.dram_tensor("cr", [128, 256], fp16, kind="ExternalInput")
    out_t = nc.dram_tensor("out", [ROWS, 3, 64 * 64], fp16, kind="ExternalOutput")

    evac_ct = [0]

    def evac(dst, src):
        # Alternate PSUM evacuation between ScalarE and VectorE
        if evac_ct[0] % 2 == 0:
            nc.scalar.copy(dst, src)
        else:
            nc.vector.tensor_copy(dst, src)
        evac_ct[0] += 1

    with tile.TileContext(nc) as tc, \
         nc.allow_low_precision(reason="fp16 softmax weights; 2e-2 tolerance"):
      for _rep in range(KREP):
       with ExitStack() as ctx:
        cpool = ctx.enter_context(tc.tile_pool(name="consts", bufs=1))
        cwt = cpool.tile([128, 704], fp16)
        nc.sync.dma_start(cwt[:], cw_t[:])
        crt = cpool.tile([128, 256], fp16)
        nc.sync.dma_start(crt[:], cr_t[:])
        ebias = cpool.tile([128, 1], f32)
        nc.vector.memset(ebias[:], EXP_BIAS)

        mpool = ctx.enter_context(tc.tile_pool(name="stacks", bufs=1))
        mlin0 = mpool.tile([128, 66 * 66], fp16)
        mlin1 = mpool.tile([32, 66 * 66], fp16)
        mset1 = mpool.tile([128, 66 * 66], fp16)
        mset2 = mpool.tile([128, 66 * 66], fp16)
        qstack = mpool.tile([128, 64 * 64], fp16)

        # ---------------- moving conv ----------------
        # local moving_emb planes 0..9 (66x66), chunk = 6 w' x 66 d' = 396
        with tc.tile_pool(name="xm", bufs=2) as xmp, \
             tc.tile_pool(name="cps", bufs=3, space="PSUM") as cpsp:
            for a0 in (0, 2, 4, 6, 8):
                xt = xmp.tile([128, 68 * 68], fp16)
                nc.sync.dma_start(
                    xt[:], xmov[a0:a0 + 4, :, :].rearrange("q c n -> (q c) n"))
                x3 = xt[:].rearrange("p (a b) -> p a b", b=68)
                for r0 in (0, 4, 8):
                    chunks = list(range(r0, min(r0 + 4, 11)))
                    ps = cpsp.tile([128, 512], f32)
                    for jk in range(9):
                        j, k = jk // 3, jk % 3
                        for gi, cidx in enumerate(chunks):
                            w0 = cidx * 6
                            rhs = x3[:, w0 + j:w0 + j + 6, k:k + 66]
                            nc.tensor.matmul(
                                ps[32 * gi:32 * gi + 32, :396],
                                lhsT=cwt[:, 288 + jk * 32:288 + jk * 32 + 32],
                                rhs=rhs,
                                start=(jk == 0), stop=(jk == 8),
                                tile_position=(0, 32 * gi))
                    # direct per-chunk evac: psum [32,396] (both parities,
                    # 32-aligned) -> final fp16 layout
                    for gi, cidx in enumerate(chunks):
                        if a0 < 8:
                            dst = mlin0[16 * a0:16 * a0 + 32,
                                        cidx * 396:(cidx + 1) * 396]
                        else:
                            dst = mlin1[0:32, cidx * 396:(cidx + 1) * 396]
                        evac(dst, ps[32 * gi:32 * gi + 32, :396])

        # moving stacks, partition-shifted: mset_i[16h+e] = plane (h+i)
        nc.sync.dma_start(mset1[0:112, :], mlin0[16:128, :])
        nc.sync.dma_start(mset1[112:128, :], mlin1[0:16, :])
        nc.sync.dma_start(mset2[0:96, :], mlin0[32:128, :])
        nc.sync.dma_start(mset2[96:128, :], mlin1[0:32, :])

        # ---------------- fixed conv ----------------
        # rows 0..7 (64x64), chunk = 8 w x 64 d = 512
        with tc.tile_pool(name="xf", bufs=2) as xfp, \
             tc.tile_pool(name="cps2", bufs=3, space="PSUM") as cpsp2:
            for h0 in (0, 2, 4, 6):
                xt = xfp.tile([128, 66 * 66], fp16)
                nc.sync.dma_start(
                    xt[:], xfix[h0:h0 + 4, :, :].rearrange("q c n -> (q c) n"))
                x3 = xt[:].rearrange("p (a b) -> p a b", b=66)
                for r0 in (0, 4):
                    ps = cpsp2.tile([128, 512], f32)
                    for jk in range(9):
                        j, k = jk // 3, jk % 3
                        for gi in range(4):
                            w0 = (r0 + gi) * 8
                            rhs = x3[:, w0 + j:w0 + j + 8, k:k + 64]
                            nc.tensor.matmul(
                                ps[32 * gi:32 * gi + 32, :],
                                lhsT=cwt[:, jk * 32:jk * 32 + 32],
                                rhs=rhs,
                                start=(jk == 0), stop=(jk == 8),
                                tile_position=(0, 32 * gi))
                    for gi in range(4):
                        cidx = r0 + gi
                        evac(qstack[16 * h0:16 * h0 + 32,
                                    cidx * 512:(cidx + 1) * 512],
                             ps[32 * gi:32 * gi + 32, :])

        # ---------------- attention ----------------
        apool = ctx.enter_context(tc.tile_pool(name="tmul", bufs=2))
        epool = ctx.enter_context(tc.tile_pool(name="etile", bufs=8))
        npool = ctx.enter_context(tc.tile_pool(name="ndsb", bufs=2))
        rpool = ctx.enter_context(tc.tile_pool(name="recd", bufs=2))
        opool = ctx.enter_context(tc.tile_pool(name="outb", bufs=1))
        out1 = opool.tile([8, 3 * 64 * 64], fp16)
        msets = (mlin0, mset1, mset2)
        mul_ct = 0

        with tc.tile_pool(name="s4", bufs=4, space="PSUM") as s4p, \
             tc.tile_pool(name="nd", bufs=4, space="PSUM") as ndp:
            for half in (0, 1):
                fo = half * 2048
                e_tiles = {}
                for G in (0, 1):
                    npg = 16 if G == 0 else 11
                    s4_tiles = [s4p.tile([128, 512], f32, name=f"s4_{half}_{G}_{ci}", tag="s4")
                                for ci in range(4)]
                    if G == 1:
                        # partitions 112:128 get no matmul write (idx 12..15
                        # absent); zero them so exp(stale psum) can't inf out
                        for ci in range(4):
                            nc.vector.memset(s4_tiles[ci][96:128, :], 0.0)
                    for idx in range(npg):
                        p = G * 16 + idx
                        i, j, k = p // 9, (p // 3) % 3, p % 3
                        m3 = msets[i][:].rearrange("p (a b) -> p a b", b=66)
                        msrc = m3[:, half * 32 + j:half * 32 + j + 32, k:k + 64]
                        t = apool.tile([128, 2048], fp16, name="tmul", tag="t")
                        eng = nc.gpsimd if (mul_ct % GP_EVERY == GP_EVERY - 1) \
                            else nc.vector
                        eng.tensor_mul(t[:], qstack[:, fo:fo + 2048], msrc)
                        mul_ct += 1
                        a, wv = idx % 4, idx // 4
                        last_w = (npg - 1 - a) // 4
                        for ci in range(4):
                            nc.tensor.matmul(
                                s4_tiles[ci][32 * a:32 * a + 32, :],
                                lhsT=cwt[:, 576 + 32 * wv:608 + 32 * wv],
                                rhs=t[:, ci * 512:(ci + 1) * 512],
                                start=(wv == 0), stop=(wv == last_w),
                                tile_position=(0, 32 * a))
                    for ci in range(4):
                        e = epool.tile([128, 512], fp16, name=f"e_{half}_{G}_{ci}", tag="e")
                        nc.scalar.activation(e[:], s4_tiles[ci][:], Exp, bias=ebias[:])
                        e_tiles[(G, ci)] = e
                for ci in range(4):
                    # single matmul per G packs all 4 r at 32-partition stride
                    nd = ndp.tile([128, 512], f32, name=f"nd_{half}_{ci}", tag="nd")
                    for G in (0, 1):
                        nc.tensor.matmul(
                            nd[:, :],
                            lhsT=crt[:, G * 128:G * 128 + 128],
                            rhs=e_tiles[(G, ci)][:],
                            start=(G == 0), stop=(G == 1))
                    rec = rpool.tile([8, 512], fp16, name="recd", tag="rec")
                    nc.vector.reciprocal(rec[:], nd[96:104, :])
                    for r in range(3):
                        nc.vector.tensor_mul(
                            out1[:, r * 4096 + fo + ci * 512:
                                 r * 4096 + fo + (ci + 1) * 512],
                            nd[32 * r:32 * r + 8, :], rec[:])

        nc.sync.dma_start(out_t[:].rearrange("h r n -> h (r n)"), out1[:])

    nc.compile()
    return nc


def _slabs(feat_moving, feat_fixed):
    fm = np.asarray(feat_moving, np.float32)[0]   # [C, H, W, D]
    ff = np.asarray(feat_fixed, np.float32)[0]
    fixp = np.zeros((C, 66, 66, 66), np.float16)
    fixp[:, 1:65, 1:65, 1:65] = ff
    mp = np.pad(fm, ((0, 0), (1, 1), (1, 1), (1, 1)), mode="edge")
    movpp = np.zeros((C, 68, 68, 68), np.float16)
    movpp[:, 1:67, 1:67, 1:67] = mp
    xf, xm = [], []
    for m in range(NCORES):
        xf.append(np.ascontiguousarray(
            fixp[:, 8 * m:8 * m + 10].reshape(C, 10, 66 * 66).transpose(1, 0, 2)))
        xm.append(np.ascontiguousarray(
            movpp[:, 8 * m:8 * m + 12].reshape(C, 12, 68 * 68).transpose(1, 0, 2)))
    return xf, xm


def kernel(feat_moving, feat_fixed, conv_w, conv_b):
    from concourse.bass_utils import run_bass_kernel_spmd

    if "nc" not in _PROG_CACHE:
        _PROG_CACHE["nc"] = _trace_program()
    nc = _PROG_CACHE["nc"]

    cw, cr = _host_consts(np.asarray(conv_w, np.float32),
                          np.asarray(conv_b, np.float32))
    xf, xm = _slabs(feat_moving, feat_fixed)
    in_maps = [{"xfix": xf[m], "xmov": xm[m], "cw": cw, "cr": cr}
               for m in range(NCORES)]
    res = run_bass_kernel_spmd(nc, in_maps, list(range(NCORES)))
    out = np.empty((1, 3, 64, 64, 64), np.float32)
    for m in range(NCORES):
        out[0, :, 8 * m:8 * m + 8] = res.results[m]["out"].astype(np.float32).reshape(
            8, 3, 64, 64).transpose(1, 0, 2, 3)
    return out


# revision 18
# speedup vs baseline: 1.3779x; 1.3779x over previous
"""Trainium2 Bass kernel for DPBlockVFAStandard (3D local cross-attention
displacement field).

Computation (B=1, C=32, E=16, H=W=D=64):
  fixed_emb  = conv3d(feat_fixed, w, b, pad=1)                    [E,64,64,64]
  moving_emb = conv3d(edge_pad(feat_moving,1), w, b, pad=1)       [E,66,66,66]
  scores[p](h,w,d) = <fixed_emb(h,w,d), moving_emb(h+i,w+j,d+k)>/4, p=(i,j,k)
  attn = softmax_p(scores);  disp_r = sum_p attn_p * R[p,r]       [3,64,64,64]

Sharding: H split into 8 slabs of 8 rows, one per NeuronCore; halo handled
host-side by overlapping input slabs (no collectives).

Per-core device pipeline (all matmuls fp16 -> 1 cycle/row on the PE):
  - convs use a channel-split contraction: K = 16 channels x 8 planes, so one
    PSUM tile accumulates 18 fp16 matmuls (9 taps x 2 channel halves) and
    yields SIX output planes ([96, chunk]); a trailing 4-plane window covers
    the leftover planes.  PSUM evacuated directly (Act/DVE fp32->fp16 copies)
    into the final plane-stack layout [16h+e, (w,d)].
  - moving planes staged as 3 partition-shifted stacks (fp16 SBUF->SBUF DMA)
  - issue order interleaves fixed-conv halves with attention score-product
    (t = q*m) production on DVE/Pool, so the PE never starves:
      moving conv | fixed conv w-half-0 | [issue t-muls half-0]
      fixed conv w-half-1 | [issue t-muls half-1] | attn half-0 | attn half-1
  - scores: 27 elementwise fp16 muls per half + block-diag fp16 reduce
    matmuls packing 16 offsets per PSUM tile [ (a,w,h) -> 32a+8w+h ]
  - exp on ScalarE (bias -4) -> fp16; R-codebook reduction as one fp16 matmul
    per (G, chunk) with r at 32-partition stride -> aligned PSUM consumers
  - reciprocal + muls on DVE straight from PSUM; fp16 output, host upcasts
"""

import os

import numpy as np

EMBED = 16
C = 32
H = 64
NCORES = 8
ROWS = H // NCORES          # 8 output rows per core
TEMP = 4.0
EXP_BIAS = -4.0

GP_EVERY = int(os.environ.get("GP_EVERY", "3"))   # every Nth score mul -> Pool
TBUFS = int(os.environ.get("TBUFS", "16"))        # t-tile backlog depth
KREP = int(os.environ.get("KREP", "1"))           # body repetitions (timing probe)
INTERLEAVE = int(os.environ.get("INTERLEAVE", "1"))  # overlap fixed conv w/ t-muls

# cw column map
MOV_CS = 0          # moving c-split blocks: (jk*2+kap)*96, 18 x [128,96]
FIX_CS = 1728       # fixed c-split blocks
FIX_PW = 3456       # fixed pair-window blocks: jk*32, 9 x [128,32]
LRED = 3744         # 4 variants [128,32]
CWCOLS = 3872

_PROG_CACHE = {}


def _radial():
    c = np.array([-1.0, 0.0, 1.0], np.float32)
    R = np.zeros((27, 3), np.float32)
    for p in range(27):
        i, j, k = p // 9, (p // 3) % 3, p % 3
        R[p] = (c[i], c[j], c[k])
    return R


def _host_consts(conv_w, conv_b):
    """Build packed lhsT constant matrices (fp16).

    c-split block (kap, j, k) of weights wa: [128, 96] with
      row (16*pl + cc), col (16*oo + e) = wa[e, 16*kap+cc, pl-oo, j, k]
      for 0 <= pl-oo <= 2 (pl: window plane 0..7, oo: output plane 0..5)
    pair-window block (j,k): [128, 32]: cols 0:16 out-parity0 (K blocks 0..2),
      cols 16:32 parity1 (K blocks 1..3)
    LRED variant w: [128, 32], col (8w+h) sums partitions (16h..16h+16)
    cr [128, 256] fp16: blocks G of [128,128]:
      row (32a+8w+h), col (32r+h') = ind(h==h')*wr(p), p=G*16+4w+a,
      wr = R[p,r] for r<3 else 1 (r=3 -> denominator)
    """
    w = conv_w.astype(np.float32)          # [E, C, 3, 3, 3]
    wm = w / TEMP
    cw = np.zeros((128, CWCOLS), np.float32)

    def cs_block(wa, kap, j, k):
        M = np.zeros((128, 96), np.float32)
        for oo in range(6):
            for i in range(3):
                pl = oo + i
                # rows 16*pl .. +16 (cc), cols 16*oo .. +16 (e)
                M[16 * pl:16 * pl + 16, 16 * oo:16 * oo + 16] = \
                    wa[:, 16 * kap:16 * kap + 16, i, j, k].T
        return M

    for jk in range(9):
        j, k = jk // 3, jk % 3
        for kap in range(2):
            cw[:, MOV_CS + (jk * 2 + kap) * 96:MOV_CS + (jk * 2 + kap) * 96 + 96] = \
                cs_block(wm, kap, j, k)
            cw[:, FIX_CS + (jk * 2 + kap) * 96:FIX_CS + (jk * 2 + kap) * 96 + 96] = \
                cs_block(w, kap, j, k)
        for i in range(3):
            blk = w[:, :, i, j, k].T       # [C, E]
            cw[32 * i:32 * i + 32, FIX_PW + jk * 32:FIX_PW + jk * 32 + 16] = blk
            cw[32 * (i + 1):32 * (i + 1) + 32,
               FIX_PW + jk * 32 + 16:FIX_PW + jk * 32 + 32] = blk
    for wv in range(4):
        base = LRED + 32 * wv
        for h in range(8):
            cw[16 * h:16 * h + 16, base + 8 * wv + h] = 1.0

    R = _radial()
    cr = np.zeros((128, 256), np.float32)
    for G in range(2):
        npg = 16 if G == 0 else 11
        for idx in range(npg):
            p = G * 16 + idx
            a, wv = idx % 4, idx // 4
            for r in range(4):
                val = R[p, r] if r < 3 else 1.0
                for h in range(8):
                    cr[32 * a + 8 * wv + h, G * 128 + 32 * r + h] = val
    return cw.astype(np.float16), cr.astype(np.float16)


def _trace_program():
    import concourse.bacc as bacc
    import concourse.tile as tile
    import concourse.mybir as mybir
    from contextlib import ExitStack

    f32 = mybir.dt.float32
    fp16 = mybir.dt.float16
    Exp = mybir.ActivationFunctionType.Exp

    nc = bacc.Bacc("TRN2", target_bir_lowering=False, debug=False,
                   enable_asserts=True, num_devices=NCORES)
    xfix = nc.dram_tensor("xfix", [2, 10, 16, 66 * 66], fp16, kind="ExternalInput")
    xmov = nc.dram_tensor("xmov", [2, 12, 16, 68 * 68], fp16, kind="ExternalInput")
    xfpw = nc.dram_tensor("xfpw", [4, C, 66 * 66], fp16, kind="ExternalInput")
    cw_t = nc.dram_tensor("cw", [128, CWCOLS], fp16, kind="ExternalInput")
    cr_t = nc.dram_tensor("cr", [128, 256], fp16, kind="ExternalInput")
    out_t = nc.dram_tensor("out", [ROWS, 3, 64 * 64], fp16, kind="ExternalOutput")

    evac_ct = [0]

    def evac(dst, src, eng=None):
        if eng == "act" or (eng is None and evac_ct[0] % 2 == 0):
            nc.scalar.copy(dst, src)
        else:
            nc.vector.tensor_copy(dst, src)
        evac_ct[0] += 1

    with tile.TileContext(nc) as tc, \
         nc.allow_low_precision(reason="fp16 softmax weights; 2e-2 tolerance"):
      for _rep in range(KREP):
       with ExitStack() as ctx:
        cpool = ctx.enter_context(tc.tile_pool(name="consts", bufs=1))
        cwt = cpool.tile([128, CWCOLS], fp16)
        nc.sync.dma_start(cwt[:], cw_t[:])
        crt = cpool.tile([128, 256], fp16)
        nc.sync.dma_start(crt[:], cr_t[:])
        ebias = cpool.tile([128, 1], f32)
        nc.vector.memset(ebias[:], EXP_BIAS)

        mpool = ctx.enter_context(tc.tile_pool(name="stacks", bufs=1))
        mlin0 = mpool.tile([128, 66 * 66], fp16)
        mlin1 = mpool.tile([32, 66 * 66], fp16)
        mset1 = mpool.tile([128, 66 * 66], fp16)
        mset2 = mpool.tile([128, 66 * 66], fp16)
        qh = [mpool.tile([128, 2048], fp16, name=f"qh{i}") for i in range(2)]

        # ---------------- moving conv (c-split windows) ----------------
        # window W0: input planes W0..W0+7, outputs m-planes W0+oo, oo=0..5
        with tc.tile_pool(name="xm", bufs=4) as xmp, \
             tc.tile_pool(name="cps", bufs=3, space="PSUM") as cpsp:
            for W0 in (0, 4):
                xts = []
                for kap in range(2):
                    xt = xmp.tile([128, 68 * 68], fp16, name=f"xm_{W0}_{kap}",
                                  tag="xm")
                    nc.sync.dma_start(
                        xt[:], xmov[kap, W0:W0 + 8, :, :]
                        .rearrange("q c n -> (q c) n"))
                    xts.append(xt[:].rearrange("p (a b) -> p a b", b=68))
                for cidx in range(11):
                    w0 = cidx * 6
                    ps = cpsp.tile([96, 512], f32, name=f"psm_{W0}_{cidx}",
                                   tag="psm")
                    for jk in range(9):
                        j, k = jk // 3, jk % 3
                        for kap in range(2):
                            rhs = xts[kap][:, w0 + j:w0 + j + 6, k:k + 66]
                            nc.tensor.matmul(
                                ps[:, :396],
                                lhsT=cwt[:, MOV_CS + (jk * 2 + kap) * 96:
                                         MOV_CS + (jk * 2 + kap) * 96 + 96],
                                rhs=rhs,
                                start=(jk == 0 and kap == 0),
                                stop=(jk == 8 and kap == 1))
                    cs = slice(cidx * 396, (cidx + 1) * 396)
                    if W0 == 0:
                        evac(mlin0[0:96, cs], ps[0:96, :396])
                    else:
                        evac(mlin0[96:128, cs], ps[32:64, :396])
                        evac(mlin1[0:32, cs], ps[64:96, :396])

        # moving stacks, partition-shifted: mset_i[16h+e] = plane (h+i)
        nc.sync.dma_start(mset1[0:112, :], mlin0[16:128, :])
        nc.sync.dma_start(mset1[112:128, :], mlin1[0:16, :])
        nc.sync.dma_start(mset2[0:96, :], mlin0[32:128, :])
        nc.sync.dma_start(mset2[96:128, :], mlin1[0:32, :])

        # ---------------- fixed conv + attention (interleaved) ----------
        xfp = ctx.enter_context(tc.tile_pool(name="xf", bufs=3))
        cpsp2 = ctx.enter_context(tc.tile_pool(name="cps2", bufs=2, space="PSUM"))
        apool = ctx.enter_context(tc.tile_pool(name="tmul", bufs=TBUFS))
        epool = ctx.enter_context(tc.tile_pool(name="etile", bufs=8))
        rpool = ctx.enter_context(tc.tile_pool(name="recd", bufs=2))
        opool = ctx.enter_context(tc.tile_pool(name="outb", bufs=1))
        s4p = ctx.enter_context(tc.tile_pool(name="s4", bufs=4, space="PSUM"))
        ndp = ctx.enter_context(tc.tile_pool(name="nd", bufs=2, space="PSUM"))
        out1 = opool.tile([8, 3 * 64 * 64], fp16)
        msets = (mlin0, mset1, mset2)
        mul_ct = [0]

        # fixed conv inputs (full area; w1 c-split tiles + w2 pair tile)
        xf1 = []
        for kap in range(2):
            xt = xfp.tile([128, 66 * 66], fp16, name=f"xf1_{kap}", tag="xf")
            nc.sync.dma_start(
                xt[:], xfix[kap, 0:8, :, :].rearrange("q c n -> (q c) n"))
            xf1.append(xt[:].rearrange("p (a b) -> p a b", b=66))
        xt = xfp.tile([128, 66 * 66], fp16, name="xf2", tag="xf")
        nc.sync.dma_start(xt[:], xfpw[:].rearrange("q c n -> (q c) n"))
        xf2 = xt[:].rearrange("p (a b) -> p a b", b=66)

        def fixed_conv_half(half, eng):
            # q-planes 0..5 via c-split window [0-7]; 6,7 via pair window [6-9]
            for cidx in range(half * 4, half * 4 + 4):
                w0 = cidx * 8
                ps = cpsp2.tile([96, 512], f32, name=f"psf1_{cidx}", tag="psf")
                for jk in range(9):
                    j, k = jk // 3, jk % 3
                    for kap in range(2):
                        rhs = xf1[kap][:, w0 + j:w0 + j + 8, k:k + 64]
                        nc.tensor.matmul(
                            ps[:, :],
                            lhsT=cwt[:, FIX_CS + (jk * 2 + kap) * 96:
                                     FIX_CS + (jk * 2 + kap) * 96 + 96],
                            rhs=rhs,
                            start=(jk == 0 and kap == 0),
                            stop=(jk == 8 and kap == 1))
                cs = slice((cidx - half * 4) * 512, (cidx - half * 4) * 512 + 512)
                evac(qh[half][0:96, cs], ps[0:96, :], eng=eng)
                ps2 = cpsp2.tile([32, 512], f32, name=f"psf2_{cidx}", tag="psf")
                for jk in range(9):
                    j, k = jk // 3, jk % 3
                    rhs = xf2[:, w0 + j:w0 + j + 8, k:k + 64]
                    nc.tensor.matmul(
                        ps2[:, :],
                        lhsT=cwt[:, FIX_PW + jk * 32:FIX_PW + jk * 32 + 32],
                        rhs=rhs,
                        start=(jk == 0), stop=(jk == 8))
                evac(qh[half][96:128, cs], ps2[0:32, :], eng=eng)

        def issue_tmuls(half):
            tlist = []
            for G in (0, 1):
                npg = 16 if G == 0 else 11
                for idx in range(npg):
                    p = G * 16 + idx
                    i, j, k = p // 9, (p // 3) % 3, p % 3
                    m3 = msets[i][:].rearrange("p (a b) -> p a b", b=66)
                    msrc = m3[:, half * 32 + j:half * 32 + j + 32, k:k + 64]
                    t = apool.tile([128, 2048], fp16, name=f"t_{half}_{p}", tag="t")
                    eng = nc.gpsimd if (mul_ct[0] % GP_EVERY == GP_EVERY - 1) \
                        else nc.vector
                    eng.tensor_mul(t[:], qh[half][:], msrc)
                    mul_ct[0] += 1
                    tlist.append(t)
            return tlist

        def attn_half(half, tlist):
            fo = half * 2048
            e_tiles = {}
            ti = 0
            for G in (0, 1):
                npg = 16 if G == 0 else 11
                s4_tiles = [s4p.tile([128, 512], f32, name=f"s4_{half}_{G}_{ci}",
                                     tag="s4") for ci in range(4)]
                if G == 1:
                    # partitions 112:128 get no matmul write; zero them so
                    # exp(stale psum) can't inf out
                    for ci in range(4):
                        nc.vector.memset(s4_tiles[ci][96:128, :], 0.0)
                for idx in range(npg):
                    t = tlist[ti]
                    ti += 1
                    a, wv = idx % 4, idx // 4
                    last_w = (npg - 1 - a) // 4
                    for ci in range(4):
                        nc.tensor.matmul(
                            s4_tiles[ci][32 * a:32 * a + 32, :],
                            lhsT=cwt[:, LRED + 32 * wv:LRED + 32 * wv + 32],
                            rhs=t[:, ci * 512:(ci + 1) * 512],
                            start=(wv == 0), stop=(wv == last_w),
                            tile_position=(0, 32 * a))
                for ci in range(4):
                    e = epool.tile([128, 512], fp16, name=f"e_{half}_{G}_{ci}",
                                   tag="e")
                    nc.scalar.activation(e[:], s4_tiles[ci][:], Exp, bias=ebias[:])
                    e_tiles[(G, ci)] = e
            for ci in range(4):
                nd = ndp.tile([128, 512], f32, name=f"nd_{half}_{ci}", tag="nd")
                for G in (0, 1):
                    nc.tensor.matmul(
                        nd[:, :],
                        lhsT=crt[:, G * 128:G * 128 + 128],
                        rhs=e_tiles[(G, ci)][:],
                        start=(G == 0), stop=(G == 1))
                rec = rpool.tile([8, 512], fp16, name="recd", tag="rec")
                nc.vector.reciprocal(rec[:], nd[96:104, :])
                for r in range(3):
                    nc.vector.tensor_mul(
                        out1[:, r * 4096 + fo + ci * 512:
                             r * 4096 + fo + (ci + 1) * 512],
                        nd[32 * r:32 * r + 8, :], rec[:])

        if INTERLEAVE:
            fixed_conv_half(0, None)   # PE: fixed w-half-0; evacs Act/DVE
            t0 = issue_tmuls(0)        # DVE/Pool: half-0 products (overlap next)
            fixed_conv_half(1, "act")  # PE: fixed w-half-1; evacs Act only
            t1 = issue_tmuls(1)        # DVE/Pool: half-1 products
            attn_half(0, t0)
            attn_half(1, t1)
        else:
            fixed_conv_half(0, None)
            fixed_conv_half(1, None)
            t0 = issue_tmuls(0)
            attn_half(0, t0)
            t1 = issue_tmuls(1)
            attn_half(1, t1)

        nc.sync.dma_start(out_t[:].rearrange("h r n -> h (r n)"), out1[:])

    nc.compile()
    return nc


def _slabs(feat_moving, feat_fixed):
    fm = np.asarray(feat_moving, np.float32)[0]   # [C, H, W, D]
    ff = np.asarray(feat_fixed, np.float32)[0]
    fixp = np.zeros((C, 66, 66, 66), np.float16)
    fixp[:, 1:65, 1:65, 1:65] = ff
    mp = np.pad(fm, ((0, 0), (1, 1), (1, 1), (1, 1)), mode="edge")
    movpp = np.zeros((C, 68, 68, 68), np.float16)
    movpp[:, 1:67, 1:67, 1:67] = mp
    xf, xm, xfw = [], [], []
    for m in range(NCORES):
        f = fixp[:, 8 * m:8 * m + 10].reshape(2, 16, 10, 66 * 66).transpose(0, 2, 1, 3)
        xf.append(np.ascontiguousarray(f))
        fw = fixp[:, 8 * m + 6:8 * m + 10].reshape(C, 4, 66 * 66).transpose(1, 0, 2)
        xfw.append(np.ascontiguousarray(fw))
        v = movpp[:, 8 * m:8 * m + 12].reshape(2, 16, 12, 68 * 68).transpose(0, 2, 1, 3)
        xm.append(np.ascontiguousarray(v))
    return xf, xm, xfw


def kernel(feat_moving, feat_fixed, conv_w, conv_b):
    from concourse.bass_utils import run_bass_kernel_spmd

    if "nc" not in _PROG_CACHE:
        _PROG_CACHE["nc"] = _trace_program()
    nc = _PROG_CACHE["nc"]

    cw, cr = _host_consts(np.asarray(conv_w, np.float32),
                          np.asarray(conv_b, np.float32))
    xf, xm, xfw = _slabs(feat_moving, feat_fixed)
    in_maps = [{"xfix": xf[m], "xmov": xm[m], "xfpw": xfw[m], "cw": cw, "cr": cr}
               for m in range(NCORES)]
    res = run_bass_kernel_spmd(nc, in_maps, list(range(NCORES)))
    out = np.empty((1, 3, 64, 64, 64), np.float32)
    for m in range(NCORES):
        out[0, :, 8 * m:8 * m + 8] = res.results[m]["out"].astype(np.float32).reshape(
            8, 3, 64, 64).transpose(1, 0, 2, 3)
    return out


# revision 19
# speedup vs baseline: 1.4762x; 1.0713x over previous
"""Trainium2 Bass kernel for DPBlockVFAStandard (3D local cross-attention
displacement field).

Computation (B=1, C=32, E=16, H=W=D=64):
  fixed_emb  = conv3d(feat_fixed, w, b, pad=1)                    [E,64,64,64]
  moving_emb = conv3d(edge_pad(feat_moving,1), w, b, pad=1)       [E,66,66,66]
  scores[p](h,w,d) = <fixed_emb(h,w,d), moving_emb(h+i,w+j,d+k)>/4, p=(i,j,k)
  attn = softmax_p(scores);  disp_r = sum_p attn_p * R[p,r]       [3,64,64,64]

Sharding: H split into 8 slabs of 8 rows, one per NeuronCore; halo handled
host-side by overlapping input slabs (no collectives).

Per-core device pipeline (all matmuls fp16 -> 1 cycle/row on the PE):
  - convs use a channel-split contraction: K = 16 channels x 8 planes, so one
    PSUM tile accumulates 18 fp16 matmuls (9 taps x 2 channel halves) and
    yields SIX output planes ([96, chunk]); a trailing 4-plane window covers
    the leftover planes.  PSUM evacuated directly (Act/DVE fp32->fp16 copies)
    into the final plane-stack layout [16h+e, (w,d)].
  - moving planes staged as 3 partition-shifted stacks (fp16 SBUF->SBUF DMA)
  - issue order interleaves fixed-conv halves with attention score-product
    (t = q*m) production on DVE/Pool, so the PE never starves:
      moving conv | fixed conv w-half-0 | [issue t-muls half-0]
      fixed conv w-half-1 | [issue t-muls half-1] | attn half-0 | attn half-1
  - scores: 27 elementwise fp16 muls per half + block-diag fp16 reduce
    matmuls packing 16 offsets per PSUM tile [ (a,w,h) -> 32a+8w+h ]
  - exp on ScalarE (bias -4) -> fp16; R-codebook reduction as one fp16 matmul
    per (G, chunk) with r at 32-partition stride -> aligned PSUM consumers
  - reciprocal + muls on DVE straight from PSUM; fp16 output, host upcasts
"""

import os

import numpy as np

EMBED = 16
C = 32
H = 64
NCORES = 8
ROWS = H // NCORES          # 8 output rows per core
TEMP = 4.0
EXP_BIAS = -4.0

GP_EVERY = int(os.environ.get("GP_EVERY", "3"))   # every Nth score mul -> Pool
TBUFS = int(os.environ.get("TBUFS", "16"))        # t-tile backlog depth
KREP = int(os.environ.get("KREP", "1"))           # body repetitions (timing probe)
INTERLEAVE = int(os.environ.get("INTERLEAVE", "1"))  # overlap fixed conv w/ t-muls

# cw column map
MOV_CS = 0          # moving c-split blocks: (jk*2+kap)*96, 18 x [128,96]
FIX_CS = 1728       # fixed c-split blocks
FIX_PW = 3456       # fixed pair-window blocks: jk*32, 9 x [128,32]
LRED = 3744         # 4 variants [128,32]
CWCOLS = 3872

_PROG_CACHE = {}


def _radial():
    c = np.array([-1.0, 0.0, 1.0], np.float32)
    R = np.zeros((27, 3), np.float32)
    for p in range(27):
        i, j, k = p // 9, (p // 3) % 3, p % 3
        R[p] = (c[i], c[j], c[k])
    return R


def _host_consts(conv_w, conv_b):
    """Build packed lhsT constant matrices (fp16).

    c-split block (kap, j, k) of weights wa: [128, 96] with
      row (16*pl + cc), col (16*oo + e) = wa[e, 16*kap+cc, pl-oo, j, k]
      for 0 <= pl-oo <= 2 (pl: window plane 0..7, oo: output plane 0..5)
    pair-window block (j,k): [128, 32]: cols 0:16 out-parity0 (K blocks 0..2),
      cols 16:32 parity1 (K blocks 1..3)
    LRED variant w: [128, 32], col (8w+h) sums partitions (16h..16h+16)
    cr [128, 256] fp16: blocks G of [128,128]:
      row (32a+8w+h), col (32r+h') = ind(h==h')*wr(p), p=G*16+4w+a,
      wr = R[p,r] for r<3 else 1 (r=3 -> denominator)
    """
    w = conv_w.astype(np.float32)          # [E, C, 3, 3, 3]
    wm = w / TEMP
    cw = np.zeros((128, CWCOLS), np.float32)

    def cs_block(wa, kap, j, k):
        M = np.zeros((128, 96), np.float32)
        for oo in range(6):
            for i in range(3):
                pl = oo + i
                # rows 16*pl .. +16 (cc), cols 16*oo .. +16 (e)
                M[16 * pl:16 * pl + 16, 16 * oo:16 * oo + 16] = \
                    wa[:, 16 * kap:16 * kap + 16, i, j, k].T
        return M

    for jk in range(9):
        j, k = jk // 3, jk % 3
        for kap in range(2):
            cw[:, MOV_CS + (jk * 2 + kap) * 96:MOV_CS + (jk * 2 + kap) * 96 + 96] = \
                cs_block(wm, kap, j, k)
            cw[:, FIX_CS + (jk * 2 + kap) * 96:FIX_CS + (jk * 2 + kap) * 96 + 96] = \
                cs_block(w, kap, j, k)
        for i in range(3):
            blk = w[:, :, i, j, k].T       # [C, E]
            cw[32 * i:32 * i + 32, FIX_PW + jk * 32:FIX_PW + jk * 32 + 16] = blk
            cw[32 * (i + 1):32 * (i + 1) + 32,
               FIX_PW + jk * 32 + 16:FIX_PW + jk * 32 + 32] = blk
    for wv in range(4):
        base = LRED + 32 * wv
        for h in range(8):
            cw[16 * h:16 * h + 16, base + 8 * wv + h] = 1.0

    R = _radial()
    cr = np.zeros((128, 256), np.float32)
    for G in range(2):
        npg = 16 if G == 0 else 11
        for idx in range(npg):
            p = G * 16 + idx
            a, wv = idx % 4, idx // 4
            for r in range(4):
                val = R[p, r] if r < 3 else 1.0
                for h in range(8):
                    cr[32 * a + 8 * wv + h, G * 128 + 32 * r + h] = val
    return cw.astype(np.float16), cr.astype(np.float16)


def _trace_program():
    import concourse.bacc as bacc
    import concourse.tile as tile
    import concourse.mybir as mybir
    from contextlib import ExitStack

    f32 = mybir.dt.float32
    fp16 = mybir.dt.float16
    Exp = mybir.ActivationFunctionType.Exp

    nc = bacc.Bacc("TRN2", target_bir_lowering=False, debug=False,
                   enable_asserts=True, num_devices=NCORES)
    xfix = nc.dram_tensor("xfix", [2, 10, 16, 66 * 66], fp16, kind="ExternalInput")
    xmov = nc.dram_tensor("xmov", [2, 12, 16, 68 * 68], fp16, kind="ExternalInput")
    xfpw = nc.dram_tensor("xfpw", [4, C, 66 * 66], fp16, kind="ExternalInput")
    cw_t = nc.dram_tensor("cw", [128, CWCOLS], fp16, kind="ExternalInput")
    cr_t = nc.dram_tensor("cr", [128, 256], fp16, kind="ExternalInput")
    out_t = nc.dram_tensor("out", [ROWS, 3, 64 * 64], fp16, kind="ExternalOutput")

    evac_ct = [0]

    def evac(dst, src, eng=None):
        if eng == "act" or (eng is None and evac_ct[0] % 2 == 0):
            nc.scalar.copy(dst, src)
        else:
            nc.vector.tensor_copy(dst, src)
        evac_ct[0] += 1

    with tile.TileContext(nc) as tc, \
         nc.allow_low_precision(reason="fp16 softmax weights; 2e-2 tolerance"):
      for _rep in range(KREP):
       with ExitStack() as ctx:
        cpool = ctx.enter_context(tc.tile_pool(name="consts", bufs=1))
        cwm = cpool.tile([128, 1728], fp16)
        nc.sync.dma_start(cwm[:], cw_t[:, 0:1728])
        cwf = cpool.tile([128, CWCOLS - 1728], fp16)
        nc.sync.dma_start(cwf[:], cw_t[:, 1728:])
        crt = cpool.tile([128, 256], fp16)
        nc.sync.dma_start(crt[:], cr_t[:])
        ebias = cpool.tile([128, 1], f32)
        nc.vector.memset(ebias[:], EXP_BIAS)

        mpool = ctx.enter_context(tc.tile_pool(name="stacks", bufs=1))
        mlin0 = mpool.tile([128, 66 * 66], fp16)
        mlin1 = mpool.tile([32, 66 * 66], fp16)
        mset1 = mpool.tile([128, 66 * 66], fp16)
        mset2 = mpool.tile([128, 66 * 66], fp16)
        qh = [mpool.tile([128, 2048], fp16, name=f"qh{i}") for i in range(2)]

        # fixed conv inputs issued first: no deps, so the SP queue serves
        # them before the mset copies and fixed conv never waits
        xfp = ctx.enter_context(tc.tile_pool(name="xf", bufs=3))
        xf1 = []
        for kap in range(2):
            xt = xfp.tile([128, 66 * 66], fp16, name=f"xf1_{kap}", tag="xf")
            nc.sync.dma_start(
                xt[:], xfix[kap, 0:8, :, :].rearrange("q c n -> (q c) n"))
            xf1.append(xt[:].rearrange("p (a b) -> p a b", b=66))
        xt = xfp.tile([128, 66 * 66], fp16, name="xf2", tag="xf")
        nc.sync.dma_start(xt[:], xfpw[:].rearrange("q c n -> (q c) n"))
        xf2 = xt[:].rearrange("p (a b) -> p a b", b=66)

        # ---------------- moving conv (c-split windows) ----------------
        # window W0: input planes W0..W0+7, outputs m-planes W0+oo, oo=0..5
        with tc.tile_pool(name="xm", bufs=4) as xmp, \
             tc.tile_pool(name="cps", bufs=3, space="PSUM") as cpsp:
            for W0 in (0, 4):
                xts = []
                for kap in range(2):
                    xt = xmp.tile([128, 68 * 68], fp16, name=f"xm_{W0}_{kap}",
                                  tag="xm")
                    nc.sync.dma_start(
                        xt[:], xmov[kap, W0:W0 + 8, :, :]
                        .rearrange("q c n -> (q c) n"))
                    xts.append(xt[:].rearrange("p (a b) -> p a b", b=68))
                for cidx in range(11):
                    w0 = cidx * 6
                    ps = cpsp.tile([96, 512], f32, name=f"psm_{W0}_{cidx}",
                                   tag="psm")
                    for jk in range(9):
                        j, k = jk // 3, jk % 3
                        for kap in range(2):
                            rhs = xts[kap][:, w0 + j:w0 + j + 6, k:k + 66]
                            nc.tensor.matmul(
                                ps[:, :396],
                                lhsT=cwm[:, (jk * 2 + kap) * 96:
                                         (jk * 2 + kap) * 96 + 96],
                                rhs=rhs,
                                start=(jk == 0 and kap == 0),
                                stop=(jk == 8 and kap == 1))
                    cs = slice(cidx * 396, (cidx + 1) * 396)
                    if W0 == 0:
                        evac(mlin0[0:96, cs], ps[0:96, :396])
                    else:
                        evac(mlin0[96:128, cs], ps[32:64, :396], eng="act")
                        evac(mlin1[0:32, cs], ps[64:96, :396], eng="act")

        # moving stacks, partition-shifted: mset_i[16h+e] = plane (h+i)
        nc.sync.dma_start(mset1[0:112, :], mlin0[16:128, :])
        nc.sync.dma_start(mset1[112:128, :], mlin1[0:16, :])
        nc.sync.dma_start(mset2[0:96, :], mlin0[32:128, :])
        nc.sync.dma_start(mset2[96:128, :], mlin1[0:32, :])

        # ---------------- fixed conv + attention (interleaved) ----------
        cpsp2 = ctx.enter_context(tc.tile_pool(name="cps2", bufs=2, space="PSUM"))
        apool = ctx.enter_context(tc.tile_pool(name="tmul", bufs=TBUFS))
        epool = ctx.enter_context(tc.tile_pool(name="etile", bufs=8))
        rpool = ctx.enter_context(tc.tile_pool(name="recd", bufs=2))
        opool = ctx.enter_context(tc.tile_pool(name="outb", bufs=1))
        s4p = ctx.enter_context(tc.tile_pool(name="s4", bufs=4, space="PSUM"))
        ndp = ctx.enter_context(tc.tile_pool(name="nd", bufs=2, space="PSUM"))
        out1 = opool.tile([8, 3 * 64 * 64], fp16)
        msets = (mlin0, mset1, mset2)
        mul_ct = [0]

        def fixed_conv_half(half, eng):
            # q-planes 0..5 via c-split window [0-7]; 6,7 via pair window [6-9]
            for cidx in range(half * 4, half * 4 + 4):
                w0 = cidx * 8
                ps = cpsp2.tile([96, 512], f32, name=f"psf1_{cidx}", tag="psf")
                for jk in range(9):
                    j, k = jk // 3, jk % 3
                    for kap in range(2):
                        rhs = xf1[kap][:, w0 + j:w0 + j + 8, k:k + 64]
                        nc.tensor.matmul(
                            ps[:, :],
                            lhsT=cwf[:, FIX_CS - 1728 + (jk * 2 + kap) * 96:
                                     FIX_CS - 1728 + (jk * 2 + kap) * 96 + 96],
                            rhs=rhs,
                            start=(jk == 0 and kap == 0),
                            stop=(jk == 8 and kap == 1))
                cs = slice((cidx - half * 4) * 512, (cidx - half * 4) * 512 + 512)
                evac(qh[half][0:96, cs], ps[0:96, :], eng=eng)
                ps2 = cpsp2.tile([32, 512], f32, name=f"psf2_{cidx}", tag="psf")
                for jk in range(9):
                    j, k = jk // 3, jk % 3
                    rhs = xf2[:, w0 + j:w0 + j + 8, k:k + 64]
                    nc.tensor.matmul(
                        ps2[:, :],
                        lhsT=cwf[:, FIX_PW - 1728 + jk * 32:
                                 FIX_PW - 1728 + jk * 32 + 32],
                        rhs=rhs,
                        start=(jk == 0), stop=(jk == 8))
                evac(qh[half][96:128, cs], ps2[0:32, :], eng=eng)

        def issue_tmuls(half):
            tlist = []
            for G in (0, 1):
                npg = 16 if G == 0 else 11
                for idx in range(npg):
                    p = G * 16 + idx
                    i, j, k = p // 9, (p // 3) % 3, p % 3
                    m3 = msets[i][:].rearrange("p (a b) -> p a b", b=66)
                    msrc = m3[:, half * 32 + j:half * 32 + j + 32, k:k + 64]
                    t = apool.tile([128, 2048], fp16, name=f"t_{half}_{p}", tag="t")
                    eng = nc.gpsimd if (mul_ct[0] % GP_EVERY == GP_EVERY - 1) \
                        else nc.vector
                    eng.tensor_mul(t[:], qh[half][:], msrc)
                    mul_ct[0] += 1
                    tlist.append(t)
            return tlist

        def attn_half(half, tlist):
            fo = half * 2048
            e_tiles = {}
            ti = 0
            for G in (0, 1):
                npg = 16 if G == 0 else 11
                s4_tiles = [s4p.tile([128, 512], f32, name=f"s4_{half}_{G}_{ci}",
                                     tag="s4") for ci in range(4)]
                if G == 1:
                    # partitions 112:128 get no matmul write; zero them so
                    # exp(stale psum) can't inf out
                    for ci in range(4):
                        nc.vector.memset(s4_tiles[ci][96:128, :], 0.0)
                for idx in range(npg):
                    t = tlist[ti]
                    ti += 1
                    a, wv = idx % 4, idx // 4
                    last_w = (npg - 1 - a) // 4
                    for ci in range(4):
                        nc.tensor.matmul(
                            s4_tiles[ci][32 * a:32 * a + 32, :],
                            lhsT=cwf[:, LRED - 1728 + 32 * wv:
                                     LRED - 1728 + 32 * wv + 32],
                            rhs=t[:, ci * 512:(ci + 1) * 512],
                            start=(wv == 0), stop=(wv == last_w),
                            tile_position=(0, 32 * a))
                for ci in range(4):
                    e = epool.tile([128, 512], fp16, name=f"e_{half}_{G}_{ci}",
                                   tag="e")
                    nc.scalar.activation(e[:], s4_tiles[ci][:], Exp, bias=ebias[:])
                    e_tiles[(G, ci)] = e
            for ci in range(4):
                nd = ndp.tile([128, 512], f32, name=f"nd_{half}_{ci}", tag="nd")
                for G in (0, 1):
                    nc.tensor.matmul(
                        nd[:, :],
                        lhsT=crt[:, G * 128:G * 128 + 128],
                        rhs=e_tiles[(G, ci)][:],
                        start=(G == 0), stop=(G == 1))
                rec = rpool.tile([8, 512], fp16, name="recd", tag="rec")
                nc.vector.reciprocal(rec[:], nd[96:104, :])
                for r in range(3):
                    nc.vector.tensor_mul(
                        out1[:, r * 4096 + fo + ci * 512:
                             r * 4096 + fo + (ci + 1) * 512],
                        nd[32 * r:32 * r + 8, :], rec[:])

        if INTERLEAVE:
            fixed_conv_half(0, "act")  # PE: fixed w-half-0; evacs Act only
            t0 = issue_tmuls(0)        # DVE/Pool: half-0 products (overlap next)
            fixed_conv_half(1, "act")  # PE: fixed w-half-1; evacs Act only
            t1 = issue_tmuls(1)        # DVE/Pool: half-1 products
            attn_half(0, t0)
            attn_half(1, t1)
        else:
            fixed_conv_half(0, None)
            fixed_conv_half(1, None)
            t0 = issue_tmuls(0)
            attn_half(0, t0)
            t1 = issue_tmuls(1)
            attn_half(1, t1)

        nc.sync.dma_start(out_t[:].rearrange("h r n -> h (r n)"), out1[:])

    nc.compile()
    return nc


def _slabs(feat_moving, feat_fixed):
    fm = np.asarray(feat_moving, np.float32)[0]   # [C, H, W, D]
    ff = np.asarray(feat_fixed, np.float32)[0]
    fixp = np.zeros((C, 66, 66, 66), np.float16)
    fixp[:, 1:65, 1:65, 1:65] = ff
    mp = np.pad(fm, ((0, 0), (1, 1), (1, 1), (1, 1)), mode="edge")
    movpp = np.zeros((C, 68, 68, 68), np.float16)
    movpp[:, 1:67, 1:67, 1:67] = mp
    xf, xm, xfw = [], [], []
    for m in range(NCORES):
        f = fixp[:, 8 * m:8 * m + 10].reshape(2, 16, 10, 66 * 66).transpose(0, 2, 1, 3)
        xf.append(np.ascontiguousarray(f))
        fw = fixp[:, 8 * m + 6:8 * m + 10].reshape(C, 4, 66 * 66).transpose(1, 0, 2)
        xfw.append(np.ascontiguousarray(fw))
        v = movpp[:, 8 * m:8 * m + 12].reshape(2, 16, 12, 68 * 68).transpose(0, 2, 1, 3)
        xm.append(np.ascontiguousarray(v))
    return xf, xm, xfw


def kernel(feat_moving, feat_fixed, conv_w, conv_b):
    from concourse.bass_utils import run_bass_kernel_spmd

    if "nc" not in _PROG_CACHE:
        _PROG_CACHE["nc"] = _trace_program()
    nc = _PROG_CACHE["nc"]

    cw, cr = _host_consts(np.asarray(conv_w, np.float32),
                          np.asarray(conv_b, np.float32))
    xf, xm, xfw = _slabs(feat_moving, feat_fixed)
    in_maps = [{"xfix": xf[m], "xmov": xm[m], "xfpw": xfw[m], "cw": cw, "cr": cr}
               for m in range(NCORES)]
    res = run_bass_kernel_spmd(nc, in_maps, list(range(NCORES)))
    out = np.empty((1, 3, 64, 64, 64), np.float32)
    for m in range(NCORES):
        out[0, :, 8 * m:8 * m + 8] = res.results[m]["out"].astype(np.float32).reshape(
            8, 3, 64, 64).transpose(1, 0, 2, 3)
    return out


# revision 20
# speedup vs baseline: 1.5049x; 1.0195x over previous
"""Trainium2 Bass kernel for DPBlockVFAStandard (3D local cross-attention
displacement field).

Computation (B=1, C=32, E=16, H=W=D=64):
  fixed_emb  = conv3d(feat_fixed, w, b, pad=1)                    [E,64,64,64]
  moving_emb = conv3d(edge_pad(feat_moving,1), w, b, pad=1)       [E,66,66,66]
  scores[p](h,w,d) = <fixed_emb(h,w,d), moving_emb(h+i,w+j,d+k)>/4, p=(i,j,k)
  attn = softmax_p(scores);  disp_r = sum_p attn_p * R[p,r]       [3,64,64,64]

Sharding: H split into 8 slabs of 8 rows, one per NeuronCore; halo handled
host-side by overlapping input slabs (no collectives).

Per-core device pipeline (all matmuls fp16 -> 1 cycle/row on the PE):
  - convs use a channel-split contraction: K = 16 channels x 8 planes, so one
    PSUM tile accumulates 18 fp16 matmuls (9 taps x 2 channel halves) and
    yields SIX output planes ([96, chunk]); a trailing 4-plane window covers
    the leftover planes.  PSUM evacuated directly (Act/DVE fp32->fp16 copies)
    into the final plane-stack layout [16h+e, (w,d)].
  - moving planes staged as 3 partition-shifted stacks (fp16 SBUF->SBUF DMA)
  - issue order interleaves fixed-conv halves with attention score-product
    (t = q*m) production on DVE/Pool, so the PE never starves:
      moving conv | fixed conv w-half-0 | [issue t-muls half-0]
      fixed conv w-half-1 | [issue t-muls half-1] | attn half-0 | attn half-1
  - scores: 27 elementwise fp16 muls per half + block-diag fp16 reduce
    matmuls packing 16 offsets per PSUM tile [ (a,w,h) -> 32a+8w+h ]
  - exp on ScalarE (bias -4) -> fp16; R-codebook reduction as one fp16 matmul
    per (G, chunk) with r at 32-partition stride -> aligned PSUM consumers
  - reciprocal + muls on DVE straight from PSUM; fp16 output, host upcasts
"""

import os

import numpy as np

EMBED = 16
C = 32
H = 64
NCORES = 8
ROWS = H // NCORES          # 8 output rows per core
TEMP = 4.0
EXP_BIAS = -4.0

GP_EVERY = int(os.environ.get("GP_EVERY", "3"))   # every Nth score mul -> Pool
TBUFS = int(os.environ.get("TBUFS", "16"))        # t-tile backlog depth
KREP = int(os.environ.get("KREP", "1"))           # body repetitions (timing probe)
INTERLEAVE = int(os.environ.get("INTERLEAVE", "1"))  # overlap fixed conv w/ t-muls

# cw column map
MOV_CS = 0          # moving c-split blocks: (jk*2+kap)*96, 18 x [128,96]
FIX_CS = 1728       # fixed c-split blocks
FIX_PW = 3456       # fixed pair-window blocks: jk*32, 9 x [128,32]
LRED = 3744         # 4 variants [128,32]
CWCOLS = 3872

_PROG_CACHE = {}


def _radial():
    c = np.array([-1.0, 0.0, 1.0], np.float32)
    R = np.zeros((27, 3), np.float32)
    for p in range(27):
        i, j, k = p // 9, (p // 3) % 3, p % 3
        R[p] = (c[i], c[j], c[k])
    return R


def _host_consts(conv_w, conv_b):
    """Build packed lhsT constant matrices (fp16).

    c-split block (kap, j, k) of weights wa: [128, 96] with
      row (16*pl + cc), col (16*oo + e) = wa[e, 16*kap+cc, pl-oo, j, k]
      for 0 <= pl-oo <= 2 (pl: window plane 0..7, oo: output plane 0..5)
    pair-window block (j,k): [128, 32]: cols 0:16 out-parity0 (K blocks 0..2),
      cols 16:32 parity1 (K blocks 1..3)
    LRED variant w: [128, 32], col (8w+h) sums partitions (16h..16h+16)
    cr [128, 256] fp16: blocks G of [128,128]:
      row (32a+8w+h), col (32r+h') = ind(h==h')*wr(p), p=G*16+4w+a,
      wr = R[p,r] for r<3 else 1 (r=3 -> denominator)
    """
    w = conv_w.astype(np.float32)          # [E, C, 3, 3, 3]
    wm = w / TEMP
    cw = np.zeros((128, CWCOLS), np.float32)

    def cs_block(wa, kap, j, k):
        M = np.zeros((128, 96), np.float32)
        for oo in range(6):
            for i in range(3):
                pl = oo + i
                # rows 16*pl .. +16 (cc), cols 16*oo .. +16 (e)
                M[16 * pl:16 * pl + 16, 16 * oo:16 * oo + 16] = \
                    wa[:, 16 * kap:16 * kap + 16, i, j, k].T
        return M

    for jk in range(9):
        j, k = jk // 3, jk % 3
        for kap in range(2):
            cw[:, MOV_CS + (jk * 2 + kap) * 96:MOV_CS + (jk * 2 + kap) * 96 + 96] = \
                cs_block(wm, kap, j, k)
            cw[:, FIX_CS + (jk * 2 + kap) * 96:FIX_CS + (jk * 2 + kap) * 96 + 96] = \
                cs_block(w, kap, j, k)
        for i in range(3):
            blk = w[:, :, i, j, k].T       # [C, E]
            cw[32 * i:32 * i + 32, FIX_PW + jk * 32:FIX_PW + jk * 32 + 16] = blk
            cw[32 * (i + 1):32 * (i + 1) + 32,
               FIX_PW + jk * 32 + 16:FIX_PW + jk * 32 + 32] = blk
    for wv in range(4):
        base = LRED + 32 * wv
        for h in range(8):
            cw[16 * h:16 * h + 16, base + 8 * wv + h] = 1.0

    R = _radial()
    cr = np.zeros((128, 256), np.float32)
    for G in range(2):
        npg = 16 if G == 0 else 11
        for idx in range(npg):
            p = G * 16 + idx
            a, wv = idx % 4, idx // 4
            for r in range(4):
                val = R[p, r] if r < 3 else 1.0
                for h in range(8):
                    cr[32 * a + 8 * wv + h, G * 128 + 32 * r + h] = val
    return cw.astype(np.float16), cr.astype(np.float16)


def _trace_program():
    import concourse.bacc as bacc
    import concourse.tile as tile
    import concourse.mybir as mybir
    from contextlib import ExitStack

    f32 = mybir.dt.float32
    fp16 = mybir.dt.float16
    Exp = mybir.ActivationFunctionType.Exp

    nc = bacc.Bacc("TRN2", target_bir_lowering=False, debug=False,
                   enable_asserts=True, num_devices=NCORES)
    xfix = nc.dram_tensor("xfix", [2, 10, 16, 66 * 66], fp16, kind="ExternalInput")
    xmov = nc.dram_tensor("xmov", [2, 12, 16, 68 * 68], fp16, kind="ExternalInput")
    xfpw = nc.dram_tensor("xfpw", [4, C, 66 * 66], fp16, kind="ExternalInput")
    cw_t = nc.dram_tensor("cw", [128, CWCOLS], fp16, kind="ExternalInput")
    cr_t = nc.dram_tensor("cr", [128, 256], fp16, kind="ExternalInput")
    out_t = nc.dram_tensor("out", [ROWS, 3, 64 * 64], fp16, kind="ExternalOutput")

    evac_ct = [0]

    def evac(dst, src, eng=None):
        if eng == "act" or (eng is None and evac_ct[0] % 2 == 0):
            nc.scalar.copy(dst, src)
        else:
            nc.vector.tensor_copy(dst, src)
        evac_ct[0] += 1

    with tile.TileContext(nc) as tc, \
         nc.allow_low_precision(reason="fp16 softmax weights; 2e-2 tolerance"):
      for _rep in range(KREP):
       with ExitStack() as ctx:
        cpool = ctx.enter_context(tc.tile_pool(name="consts", bufs=1))
        cwm = cpool.tile([128, 1728], fp16)
        nc.sync.dma_start(cwm[:], cw_t[:, 0:1728])
        cwf = cpool.tile([128, CWCOLS - 1728], fp16)
        nc.sync.dma_start(cwf[:], cw_t[:, 1728:])
        crt = cpool.tile([128, 256], fp16)
        nc.sync.dma_start(crt[:], cr_t[:])
        ebias = cpool.tile([128, 1], f32)
        nc.vector.memset(ebias[:], EXP_BIAS)

        mpool = ctx.enter_context(tc.tile_pool(name="stacks", bufs=1))
        mlin0 = mpool.tile([128, 66 * 66], fp16)
        mlin1 = mpool.tile([32, 66 * 66], fp16)
        mset1 = mpool.tile([128, 66 * 66], fp16)
        mset2 = mpool.tile([128, 66 * 66], fp16)
        qh = [mpool.tile([128, 2048], fp16, name=f"qh{i}") for i in range(2)]

        # fixed conv inputs issued first: no deps, so the SP queue serves
        # them before the mset copies and fixed conv never waits
        xfp = ctx.enter_context(tc.tile_pool(name="xf", bufs=3))
        xf1 = []
        for kap in range(2):
            xt = xfp.tile([128, 66 * 66], fp16, name=f"xf1_{kap}", tag="xf")
            nc.sync.dma_start(
                xt[:], xfix[kap, 0:8, :, :].rearrange("q c n -> (q c) n"))
            xf1.append(xt[:].rearrange("p (a b) -> p a b", b=66))
        xt = xfp.tile([128, 66 * 66], fp16, name="xf2", tag="xf")
        nc.sync.dma_start(xt[:], xfpw[:].rearrange("q c n -> (q c) n"))
        xf2 = xt[:].rearrange("p (a b) -> p a b", b=66)

        # ---------------- moving conv (c-split windows) ----------------
        # window W0: input planes W0..W0+7, outputs m-planes W0+oo, oo=0..5
        with tc.tile_pool(name="xm", bufs=4) as xmp, \
             tc.tile_pool(name="cps", bufs=3, space="PSUM") as cpsp:
            for W0 in (0, 4):
                xts = []
                for kap in range(2):
                    xt = xmp.tile([128, 68 * 68], fp16, name=f"xm_{W0}_{kap}",
                                  tag="xm")
                    nc.sync.dma_start(
                        xt[:], xmov[kap, W0:W0 + 8, :, :]
                        .rearrange("q c n -> (q c) n"))
                    xts.append(xt[:].rearrange("p (a b) -> p a b", b=68))
                for cidx in range(11):
                    w0 = cidx * 6
                    ps = cpsp.tile([96, 512], f32, name=f"psm_{W0}_{cidx}",
                                   tag="psm")
                    for jk in range(9):
                        j, k = jk // 3, jk % 3
                        for kap in range(2):
                            rhs = xts[kap][:, w0 + j:w0 + j + 6, k:k + 66]
                            nc.tensor.matmul(
                                ps[:, :396],
                                lhsT=cwm[:, (jk * 2 + kap) * 96:
                                         (jk * 2 + kap) * 96 + 96],
                                rhs=rhs,
                                start=(jk == 0 and kap == 0),
                                stop=(jk == 8 and kap == 1))
                    cs = slice(cidx * 396, (cidx + 1) * 396)
                    if W0 == 0:
                        evac(mlin0[0:96, cs], ps[0:96, :396])
                    else:
                        evac(mlin0[96:128, cs], ps[32:64, :396], eng="act")
                        evac(mlin1[0:32, cs], ps[64:96, :396], eng="act")

        # moving stacks, partition-shifted: mset_i[16h+e] = plane (h+i)
        nc.sync.dma_start(mset1[0:112, :], mlin0[16:128, :])
        nc.sync.dma_start(mset1[112:128, :], mlin1[0:16, :])
        nc.sync.dma_start(mset2[0:96, :], mlin0[32:128, :])
        nc.sync.dma_start(mset2[96:128, :], mlin1[0:32, :])

        # ---------------- fixed conv + attention (interleaved) ----------
        cpsp2 = ctx.enter_context(tc.tile_pool(name="cps2", bufs=2, space="PSUM"))
        apool = ctx.enter_context(tc.tile_pool(name="tmul", bufs=TBUFS))
        epool = ctx.enter_context(tc.tile_pool(name="etile", bufs=8))
        rpool = ctx.enter_context(tc.tile_pool(name="recd", bufs=2))
        opool = ctx.enter_context(tc.tile_pool(name="outb", bufs=1))
        s4p = ctx.enter_context(tc.tile_pool(name="s4", bufs=4, space="PSUM"))
        ndp = ctx.enter_context(tc.tile_pool(name="nd", bufs=2, space="PSUM"))
        out1 = opool.tile([8, 3 * 64 * 64], fp16)
        msets = (mlin0, mset1, mset2)
        mul_ct = [0]

        def fixed_conv_half(half, eng):
            # q-planes 0..5 via c-split window [0-7]; 6,7 via pair window [6-9]
            for cidx in range(half * 4, half * 4 + 4):
                w0 = cidx * 8
                ps = cpsp2.tile([96, 512], f32, name=f"psf1_{cidx}", tag="psf")
                for jk in range(9):
                    j, k = jk // 3, jk % 3
                    for kap in range(2):
                        rhs = xf1[kap][:, w0 + j:w0 + j + 8, k:k + 64]
                        nc.tensor.matmul(
                            ps[:, :],
                            lhsT=cwf[:, FIX_CS - 1728 + (jk * 2 + kap) * 96:
                                     FIX_CS - 1728 + (jk * 2 + kap) * 96 + 96],
                            rhs=rhs,
                            start=(jk == 0 and kap == 0),
                            stop=(jk == 8 and kap == 1))
                cs = slice((cidx - half * 4) * 512, (cidx - half * 4) * 512 + 512)
                evac(qh[half][0:96, cs], ps[0:96, :], eng=eng)
                ps2 = cpsp2.tile([32, 512], f32, name=f"psf2_{cidx}", tag="psf")
                for jk in range(9):
                    j, k = jk // 3, jk % 3
                    rhs = xf2[:, w0 + j:w0 + j + 8, k:k + 64]
                    nc.tensor.matmul(
                        ps2[:, :],
                        lhsT=cwf[:, FIX_PW - 1728 + jk * 32:
                                 FIX_PW - 1728 + jk * 32 + 32],
                        rhs=rhs,
                        start=(jk == 0), stop=(jk == 8))
                evac(qh[half][96:128, cs], ps2[0:32, :], eng=eng)

        def issue_tmuls(half):
            tlist = []
            for G in (0, 1):
                npg = 16 if G == 0 else 11
                for idx in range(npg):
                    p = G * 16 + idx
                    i, j, k = p // 9, (p // 3) % 3, p % 3
                    m3 = msets[i][:].rearrange("p (a b) -> p a b", b=66)
                    msrc = m3[:, half * 32 + j:half * 32 + j + 32, k:k + 64]
                    t = apool.tile([128, 2048], fp16, name=f"t_{half}_{p}", tag="t")
                    eng = nc.gpsimd if (mul_ct[0] % GP_EVERY == GP_EVERY - 1) \
                        else nc.vector
                    eng.tensor_mul(t[:], qh[half][:], msrc)
                    mul_ct[0] += 1
                    tlist.append(t)
            return tlist

        def attn_half(half, tlist):
            fo = half * 2048
            e_tiles = {}
            ti = 0
            for G in (0, 1):
                npg = 16 if G == 0 else 11
                s4_tiles = [s4p.tile([128, 512], f32, name=f"s4_{half}_{G}_{ci}",
                                     tag="s4") for ci in range(4)]
                if G == 1:
                    # partitions 112:128 get no matmul write; zero them so
                    # exp(stale psum) can't inf out
                    for ci in range(4):
                        nc.scalar.memzero(s4_tiles[ci][96:128, :])
                for idx in range(npg):
                    t = tlist[ti]
                    ti += 1
                    a, wv = idx % 4, idx // 4
                    last_w = (npg - 1 - a) // 4
                    for ci in range(4):
                        nc.tensor.matmul(
                            s4_tiles[ci][32 * a:32 * a + 32, :],
                            lhsT=cwf[:, LRED - 1728 + 32 * wv:
                                     LRED - 1728 + 32 * wv + 32],
                            rhs=t[:, ci * 512:(ci + 1) * 512],
                            start=(wv == 0), stop=(wv == last_w),
                            tile_position=(0, 32 * a))
                for ci in range(4):
                    e = epool.tile([128, 512], fp16, name=f"e_{half}_{G}_{ci}",
                                   tag="e")
                    nc.scalar.activation(e[:], s4_tiles[ci][:], Exp, bias=ebias[:])
                    e_tiles[(G, ci)] = e
            for ci in range(4):
                nd = ndp.tile([128, 512], f32, name=f"nd_{half}_{ci}", tag="nd")
                for G in (0, 1):
                    nc.tensor.matmul(
                        nd[:, :],
                        lhsT=crt[:, G * 128:G * 128 + 128],
                        rhs=e_tiles[(G, ci)][:],
                        start=(G == 0), stop=(G == 1))
                rec = rpool.tile([8, 512], fp16, name="recd", tag="rec")
                nc.vector.reciprocal(rec[:], nd[96:104, :])
                for r in range(3):
                    nc.vector.tensor_mul(
                        out1[:, r * 4096 + fo + ci * 512:
                             r * 4096 + fo + (ci + 1) * 512],
                        nd[32 * r:32 * r + 8, :], rec[:])

        if INTERLEAVE:
            fixed_conv_half(0, "act")  # PE: fixed w-half-0; evacs Act only
            t0 = issue_tmuls(0)        # DVE/Pool: half-0 products (overlap next)
            fixed_conv_half(1, "act")  # PE: fixed w-half-1; evacs Act only
            t1 = issue_tmuls(1)        # DVE/Pool: half-1 products
            attn_half(0, t0)
            attn_half(1, t1)
        else:
            fixed_conv_half(0, None)
            fixed_conv_half(1, None)
            t0 = issue_tmuls(0)
            attn_half(0, t0)
            t1 = issue_tmuls(1)
            attn_half(1, t1)

        nc.sync.dma_start(out_t[:].rearrange("h r n -> h (r n)"), out1[:])

    nc.compile()
    return nc


def _slabs(feat_moving, feat_fixed):
    fm = np.asarray(feat_moving, np.float32)[0]   # [C, H, W, D]
    ff = np.asarray(feat_fixed, np.float32)[0]
    fixp = np.zeros((C, 66, 66, 66), np.float16)
    fixp[:, 1:65, 1:65, 1:65] = ff
    mp = np.pad(fm, ((0, 0), (1, 1), (1, 1), (1, 1)), mode="edge")
    movpp = np.zeros((C, 68, 68, 68), np.float16)
    movpp[:, 1:67, 1:67, 1:67] = mp
    xf, xm, xfw = [], [], []
    for m in range(NCORES):
        f = fixp[:, 8 * m:8 * m + 10].reshape(2, 16, 10, 66 * 66).transpose(0, 2, 1, 3)
        xf.append(np.ascontiguousarray(f))
        fw = fixp[:, 8 * m + 6:8 * m + 10].reshape(C, 4, 66 * 66).transpose(1, 0, 2)
        xfw.append(np.ascontiguousarray(fw))
        v = movpp[:, 8 * m:8 * m + 12].reshape(2, 16, 12, 68 * 68).transpose(0, 2, 1, 3)
        xm.append(np.ascontiguousarray(v))
    return xf, xm, xfw


def kernel(feat_moving, feat_fixed, conv_w, conv_b):
    from concourse.bass_utils import run_bass_kernel_spmd

    if "nc" not in _PROG_CACHE:
        _PROG_CACHE["nc"] = _trace_program()
    nc = _PROG_CACHE["nc"]

    cw, cr = _host_consts(np.asarray(conv_w, np.float32),
                          np.asarray(conv_b, np.float32))
    xf, xm, xfw = _slabs(feat_moving, feat_fixed)
    in_maps = [{"xfix": xf[m], "xmov": xm[m], "xfpw": xfw[m], "cw": cw, "cr": cr}
               for m in range(NCORES)]
    res = run_bass_kernel_spmd(nc, in_maps, list(range(NCORES)))
    out = np.empty((1, 3, 64, 64, 64), np.float32)
    for m in range(NCORES):
        out[0, :, 8 * m:8 * m + 8] = res.results[m]["out"].astype(np.float32).reshape(
            8, 3, 64, 64).transpose(1, 0, 2, 3)
    return out


# revision 23
# speedup vs baseline: 1.6055x; 1.0668x over previous
"""Trainium2 Bass kernel for DPBlockVFAStandard (3D local cross-attention
displacement field).

Computation (B=1, C=32, E=16, H=W=D=64):
  fixed_emb  = conv3d(feat_fixed, w, b, pad=1)                    [E,64,64,64]
  moving_emb = conv3d(edge_pad(feat_moving,1), w, b, pad=1)       [E,66,66,66]
  scores[p](h,w,d) = <fixed_emb(h,w,d), moving_emb(h+i,w+j,d+k)>/4, p=(i,j,k)
  attn = softmax_p(scores);  disp_r = sum_p attn_p * R[p,r]       [3,64,64,64]

Sharding: H split into 8 slabs of 8 rows, one per NeuronCore; halo handled
host-side by overlapping input slabs (no collectives).

Per-core device pipeline (all matmuls fp16 -> 1 cycle/row on the PE):
  - convs use a channel-split contraction: K = 16 channels x 8 planes, so one
    PSUM tile accumulates 18 fp16 matmuls (9 taps x 2 channel halves) and
    yields SIX output planes ([96, chunk]); a trailing 4-plane window covers
    the leftover planes.  PSUM evacuated directly (Act/DVE fp32->fp16 copies)
    into the final plane-stack layout [16h+e, (w,d)].
  - moving planes staged as 3 partition-shifted stacks (fp16 SBUF->SBUF DMA)
  - issue order interleaves fixed-conv halves with attention score-product
    (t = q*m) production on DVE/Pool, so the PE never starves:
      moving conv | fixed conv w-half-0 | [issue t-muls half-0]
      fixed conv w-half-1 | [issue t-muls half-1] | attn half-0 | attn half-1
  - scores: 27 elementwise fp16 muls per half + block-diag fp16 reduce
    matmuls packing 16 offsets per PSUM tile [ (a,w,h) -> 32a+8w+h ]
  - exp on ScalarE (bias -4) -> fp16; R-codebook reduction as one fp16 matmul
    per (G, chunk) with r at 32-partition stride -> aligned PSUM consumers
  - reciprocal + muls on DVE straight from PSUM; fp16 output, host upcasts
"""

import os

import numpy as np

EMBED = 16
C = 32
H = 64
NCORES = 8
ROWS = H // NCORES          # 8 output rows per core
TEMP = 4.0
EXP_BIAS = -4.0

GP_EVERY = int(os.environ.get("GP_EVERY", "4"))   # every Nth score mul -> Pool
TBUFS = int(os.environ.get("TBUFS", "20"))        # t-tile backlog depth
KREP = int(os.environ.get("KREP", "1"))           # body repetitions (timing probe)
INTERLEAVE = int(os.environ.get("INTERLEAVE", "1"))  # overlap fixed conv w/ t-muls

# cw column map
MOV_CS = 0          # moving c-split blocks: (jk*2+kap)*96, 18 x [128,96]
FIX_CS = 1728       # fixed c-split blocks
FIX_PW = 3456       # fixed pair-window blocks: jk*32, 9 x [128,32]
LRED = 3744         # 4 variants [128,32]
CWCOLS = 3872

_PROG_CACHE = {}


def _radial():
    c = np.array([-1.0, 0.0, 1.0], np.float32)
    R = np.zeros((27, 3), np.float32)
    for p in range(27):
        i, j, k = p // 9, (p // 3) % 3, p % 3
        R[p] = (c[i], c[j], c[k])
    return R


def _host_consts(conv_w, conv_b):
    """Build packed lhsT constant matrices (fp16).

    c-split block (kap, j, k) of weights wa: [128, 96] with
      row (16*pl + cc), col (16*oo + e) = wa[e, 16*kap+cc, pl-oo, j, k]
      for 0 <= pl-oo <= 2 (pl: window plane 0..7, oo: output plane 0..5)
    pair-window block (j,k): [128, 32]: cols 0:16 out-parity0 (K blocks 0..2),
      cols 16:32 parity1 (K blocks 1..3)
    LRED variant w: [128, 32], col (8w+h) sums partitions (16h..16h+16)
    cr [128, 256] fp16: blocks G of [128,128]:
      row (32a+8w+h), col (32r+h') = ind(h==h')*wr(p), p=G*16+4w+a,
      wr = R[p,r] for r<3 else 1 (r=3 -> denominator)
    """
    w = conv_w.astype(np.float32)          # [E, C, 3, 3, 3]
    wm = w / TEMP
    cw = np.zeros((128, CWCOLS), np.float32)

    def cs_block(wa, kap, j, k):
        M = np.zeros((128, 96), np.float32)
        for oo in range(6):
            for i in range(3):
                pl = oo + i
                # rows 16*pl .. +16 (cc), cols 16*oo .. +16 (e)
                M[16 * pl:16 * pl + 16, 16 * oo:16 * oo + 16] = \
                    wa[:, 16 * kap:16 * kap + 16, i, j, k].T
        return M

    for jk in range(9):
        j, k = jk // 3, jk % 3
        for kap in range(2):
            cw[:, MOV_CS + (jk * 2 + kap) * 96:MOV_CS + (jk * 2 + kap) * 96 + 96] = \
                cs_block(wm, kap, j, k)
            cw[:, FIX_CS + (jk * 2 + kap) * 96:FIX_CS + (jk * 2 + kap) * 96 + 96] = \
                cs_block(w, kap, j, k)
        for i in range(3):
            blk = w[:, :, i, j, k].T       # [C, E]
            cw[32 * i:32 * i + 32, FIX_PW + jk * 32:FIX_PW + jk * 32 + 16] = blk
            cw[32 * (i + 1):32 * (i + 1) + 32,
               FIX_PW + jk * 32 + 16:FIX_PW + jk * 32 + 32] = blk
    for wv in range(4):
        base = LRED + 32 * wv
        for h in range(8):
            cw[16 * h:16 * h + 16, base + 8 * wv + h] = 1.0

    R = _radial()
    cr = np.zeros((128, 256), np.float32)
    for G in range(2):
        npg = 16 if G == 0 else 11
        for idx in range(npg):
            p = G * 16 + idx
            a, wv = idx % 4, idx // 4
            for r in range(4):
                val = R[p, r] if r < 3 else 1.0
                for h in range(8):
                    cr[32 * a + 8 * wv + h, G * 128 + 32 * r + h] = val
    return cw.astype(np.float16), cr.astype(np.float16)


def _trace_program():
    import concourse.bacc as bacc
    import concourse.tile as tile
    import concourse.mybir as mybir
    from contextlib import ExitStack

    f32 = mybir.dt.float32
    fp16 = mybir.dt.float16
    Exp = mybir.ActivationFunctionType.Exp

    nc = bacc.Bacc("TRN2", target_bir_lowering=False, debug=False,
                   enable_asserts=True, num_devices=NCORES)
    xfix = nc.dram_tensor("xfix", [2, 10, 16, 66 * 66], fp16, kind="ExternalInput")
    xmov = nc.dram_tensor("xmov", [2, 12, 16, 68 * 68], fp16, kind="ExternalInput")
    xfpw = nc.dram_tensor("xfpw", [4, C, 66 * 66], fp16, kind="ExternalInput")
    cw_t = nc.dram_tensor("cw", [128, CWCOLS], fp16, kind="ExternalInput")
    cr_t = nc.dram_tensor("cr", [128, 256], fp16, kind="ExternalInput")
    out_t = nc.dram_tensor("out", [ROWS, 3, 64 * 64], fp16, kind="ExternalOutput")

    evac_ct = [0]

    def evac(dst, src, eng=None):
        if eng == "act" or (eng is None and evac_ct[0] % 2 == 0):
            nc.scalar.copy(dst, src)
        else:
            nc.vector.tensor_copy(dst, src)
        evac_ct[0] += 1

    with tile.TileContext(nc) as tc, \
         nc.allow_low_precision(reason="fp16 softmax weights; 2e-2 tolerance"):
      for _rep in range(KREP):
       with ExitStack() as ctx:
        cpool = ctx.enter_context(tc.tile_pool(name="consts", bufs=1))
        cwm = cpool.tile([128, 1728], fp16)
        nc.sync.dma_start(cwm[:], cw_t[:, 0:1728])
        cwf = cpool.tile([128, CWCOLS - 1728], fp16)
        crt = cpool.tile([128, 256], fp16)
        ebias = cpool.tile([128, 1], f32)
        nc.vector.memset(ebias[:], EXP_BIAS)

        mpool = ctx.enter_context(tc.tile_pool(name="stacks", bufs=1))
        mlin0 = mpool.tile([128, 66 * 66], fp16)
        mlin1 = mpool.tile([32, 66 * 66], fp16)
        mset1 = mpool.tile([128, 66 * 66], fp16)
        mset2 = mpool.tile([128, 66 * 66], fp16)
        qh = [mpool.tile([128, 2048], fp16, name=f"qh{i}") for i in range(2)]

        # ---------------- moving conv (c-split windows) ----------------
        # window W0: input planes W0..W0+7, outputs m-planes W0+oo, oo=0..5
        # DMA issue order: moving inputs first (the PE waits on them), then
        # the remaining consts and fixed-conv inputs fill the SP queue and
        # land well before their consumers need them.
        # pre-allocate fixed-conv input tiles (DMAs issued after moving's)
        xfp = ctx.enter_context(tc.tile_pool(name="xf", bufs=3))
        xf_tiles = [xfp.tile([128, 66 * 66], fp16, name=f"xft_{i}", tag="xf")
                    for i in range(3)]
        xf1 = [xf_tiles[i][:].rearrange("p (a b) -> p a b", b=66)
               for i in range(2)]
        xf2 = xf_tiles[2][:].rearrange("p (a b) -> p a b", b=66)

        with tc.tile_pool(name="xm", bufs=4) as xmp, \
             tc.tile_pool(name="cps", bufs=3, space="PSUM") as cpsp:
            xtv = {}
            for W0 in (0, 4):
                for kap in range(2):
                    xt = xmp.tile([128, 68 * 68], fp16, name=f"xm_{W0}_{kap}",
                                  tag="xm")
                    nc.sync.dma_start(
                        xt[:], xmov[kap, W0:W0 + 8, :, :]
                        .rearrange("q c n -> (q c) n"))
                    xtv[(W0, kap)] = xt[:].rearrange("p (a b) -> p a b", b=68)
            nc.sync.dma_start(cwf[:], cw_t[:, 1728:])
            nc.sync.dma_start(crt[:], cr_t[:])
            for kap in range(2):
                nc.sync.dma_start(
                    xf_tiles[kap][:],
                    xfix[kap, 0:8, :, :].rearrange("q c n -> (q c) n"))
            nc.sync.dma_start(xf_tiles[2][:],
                              xfpw[:].rearrange("q c n -> (q c) n"))
            for W0 in (0, 4):
                xts = [xtv[(W0, 0)], xtv[(W0, 1)]]
                for cidx in range(11):
                    w0 = cidx * 6
                    ps = cpsp.tile([96, 512], f32, name=f"psm_{W0}_{cidx}",
                                   tag="psm")
                    for jk in range(9):
                        j, k = jk // 3, jk % 3
                        for kap in range(2):
                            rhs = xts[kap][:, w0 + j:w0 + j + 6, k:k + 66]
                            nc.tensor.matmul(
                                ps[:, :396],
                                lhsT=cwm[:, (jk * 2 + kap) * 96:
                                         (jk * 2 + kap) * 96 + 96],
                                rhs=rhs,
                                start=(jk == 0 and kap == 0),
                                stop=(jk == 8 and kap == 1))
                    cs = slice(cidx * 396, (cidx + 1) * 396)
                    if W0 == 0:
                        evac(mlin0[0:96, cs], ps[0:96, :396])
                    else:
                        evac(mlin0[96:128, cs], ps[32:64, :396], eng="act")
                        evac(mlin1[0:32, cs], ps[64:96, :396], eng="act")

        # moving stacks, partition-shifted: mset_i[16h+e] = plane (h+i)
        nc.sync.dma_start(mset1[0:112, :], mlin0[16:128, :])
        nc.sync.dma_start(mset1[112:128, :], mlin1[0:16, :])
        nc.sync.dma_start(mset2[0:96, :], mlin0[32:128, :])
        nc.sync.dma_start(mset2[96:128, :], mlin1[0:32, :])

        # ---------------- fixed conv + attention (interleaved) ----------
        cpsp2 = ctx.enter_context(tc.tile_pool(name="cps2", bufs=2, space="PSUM"))
        apool = ctx.enter_context(tc.tile_pool(name="tmul", bufs=TBUFS))
        epool = ctx.enter_context(tc.tile_pool(name="etile", bufs=8))
        rpool = ctx.enter_context(tc.tile_pool(name="recd", bufs=2))
        opool = ctx.enter_context(tc.tile_pool(name="outb", bufs=1))
        s4p = ctx.enter_context(tc.tile_pool(name="s4", bufs=4, space="PSUM"))
        ndp = ctx.enter_context(tc.tile_pool(name="nd", bufs=2, space="PSUM"))
        out1 = opool.tile([8, 3 * 64 * 64], fp16)
        msets = (mlin0, mset1, mset2)
        mul_ct = [0]

        def fixed_conv_half(half, eng):
            # q-planes 0..5 via c-split window [0-7]; 6,7 via pair window [6-9]
            for cidx in range(half * 4, half * 4 + 4):
                w0 = cidx * 8
                # one [128,512] psum tile: partitions 0:96 c-split (q-planes
                # 0..5), 96:128 pair-window (q-planes 6,7) -> single evac
                ps = cpsp2.tile([128, 512], f32, name=f"psf_{cidx}", tag="psf")
                for jk in range(9):
                    j, k = jk // 3, jk % 3
                    for kap in range(2):
                        rhs = xf1[kap][:, w0 + j:w0 + j + 8, k:k + 64]
                        nc.tensor.matmul(
                            ps[0:96, :],
                            lhsT=cwf[:, FIX_CS - 1728 + (jk * 2 + kap) * 96:
                                     FIX_CS - 1728 + (jk * 2 + kap) * 96 + 96],
                            rhs=rhs,
                            start=(jk == 0 and kap == 0),
                            stop=(jk == 8 and kap == 1))
                for jk in range(9):
                    j, k = jk // 3, jk % 3
                    rhs = xf2[:, w0 + j:w0 + j + 8, k:k + 64]
                    nc.tensor.matmul(
                        ps[96:128, :],
                        lhsT=cwf[:, FIX_PW - 1728 + jk * 32:
                                 FIX_PW - 1728 + jk * 32 + 32],
                        rhs=rhs,
                        start=(jk == 0), stop=(jk == 8),
                        tile_position=(0, 96))
                cs = slice((cidx - half * 4) * 512, (cidx - half * 4) * 512 + 512)
                evac(qh[half][:, cs], ps[:, :], eng=eng)

        def issue_tmuls(half):
            tlist = []
            for G in (0, 1):
                npg = 16 if G == 0 else 11
                for idx in range(npg):
                    p = G * 16 + idx
                    i, j, k = p // 9, (p // 3) % 3, p % 3
                    m3 = msets[i][:].rearrange("p (a b) -> p a b", b=66)
                    msrc = m3[:, half * 32 + j:half * 32 + j + 32, k:k + 64]
                    t = apool.tile([128, 2048], fp16, name=f"t_{half}_{p}", tag="t")
                    eng = nc.gpsimd if (mul_ct[0] % GP_EVERY == GP_EVERY - 1) \
                        else nc.vector
                    eng.tensor_mul(t[:], qh[half][:], msrc)
                    mul_ct[0] += 1
                    tlist.append(t)
            return tlist

        def attn_half(half, tlist):
            fo = half * 2048
            e_tiles = {}
            ti = 0
            for G in (0, 1):
                npg = 16 if G == 0 else 11
                s4_tiles = [s4p.tile([128, 512], f32, name=f"s4_{half}_{G}_{ci}",
                                     tag="s4") for ci in range(4)]
                if G == 1:
                    # partitions 112:128 get no matmul write; zero them so
                    # exp(stale psum) can't inf out
                    for ci in range(4):
                        nc.scalar.memzero(s4_tiles[ci][96:128, :])
                for idx in range(npg):
                    t = tlist[ti]
                    ti += 1
                    a, wv = idx % 4, idx // 4
                    last_w = (npg - 1 - a) // 4
                    for ci in range(4):
                        nc.tensor.matmul(
                            s4_tiles[ci][32 * a:32 * a + 32, :],
                            lhsT=cwf[:, LRED - 1728 + 32 * wv:
                                     LRED - 1728 + 32 * wv + 32],
                            rhs=t[:, ci * 512:(ci + 1) * 512],
                            start=(wv == 0), stop=(wv == last_w),
                            tile_position=(0, 32 * a))
                for ci in range(4):
                    e = epool.tile([128, 512], fp16, name=f"e_{half}_{G}_{ci}",
                                   tag="e")
                    nc.scalar.activation(e[:], s4_tiles[ci][:], Exp, bias=ebias[:])
                    e_tiles[(G, ci)] = e
            for ci in range(4):
                nd = ndp.tile([128, 512], f32, name=f"nd_{half}_{ci}", tag="nd")
                for G in (0, 1):
                    nc.tensor.matmul(
                        nd[:, :],
                        lhsT=crt[:, G * 128:G * 128 + 128],
                        rhs=e_tiles[(G, ci)][:],
                        start=(G == 0), stop=(G == 1))
                rec = rpool.tile([8, 512], fp16, name="recd", tag="rec")
                nc.vector.reciprocal(rec[:], nd[96:104, :])
                for r in range(3):
                    nc.vector.tensor_mul(
                        out1[:, r * 4096 + fo + ci * 512:
                             r * 4096 + fo + (ci + 1) * 512],
                        nd[32 * r:32 * r + 8, :], rec[:])

        if INTERLEAVE:
            fixed_conv_half(0, "act")  # PE: fixed w-half-0; evacs Act only
            t0 = issue_tmuls(0)        # DVE/Pool: half-0 products (overlap next)
            fixed_conv_half(1, "act")  # PE: fixed w-half-1; evacs Act only
            t1 = issue_tmuls(1)        # DVE/Pool: half-1 products
            attn_half(0, t0)
            attn_half(1, t1)
        else:
            fixed_conv_half(0, None)
            fixed_conv_half(1, None)
            t0 = issue_tmuls(0)
            attn_half(0, t0)
            t1 = issue_tmuls(1)
            attn_half(1, t1)

        nc.sync.dma_start(out_t[:].rearrange("h r n -> h (r n)"), out1[:])

    nc.compile()
    return nc


def _slabs(feat_moving, feat_fixed):
    fm = np.asarray(feat_moving, np.float32)[0]   # [C, H, W, D]
    ff = np.asarray(feat_fixed, np.float32)[0]
    fixp = np.zeros((C, 66, 66, 66), np.float16)
    fixp[:, 1:65, 1:65, 1:65] = ff
    mp = np.pad(fm, ((0, 0), (1, 1), (1, 1), (1, 1)), mode="edge")
    movpp = np.zeros((C, 68, 68, 68), np.float16)
    movpp[:, 1:67, 1:67, 1:67] = mp
    xf, xm, xfw = [], [], []
    for m in range(NCORES):
        f = fixp[:, 8 * m:8 * m + 10].reshape(2, 16, 10, 66 * 66).transpose(0, 2, 1, 3)
        xf.append(np.ascontiguousarray(f))
        fw = fixp[:, 8 * m + 6:8 * m + 10].reshape(C, 4, 66 * 66).transpose(1, 0, 2)
        xfw.append(np.ascontiguousarray(fw))
        v = movpp[:, 8 * m:8 * m + 12].reshape(2, 16, 12, 68 * 68).transpose(0, 2, 1, 3)
        xm.append(np.ascontiguousarray(v))
    return xf, xm, xfw


def kernel(feat_moving, feat_fixed, conv_w, conv_b):
    from concourse.bass_utils import run_bass_kernel_spmd

    if "nc" not in _PROG_CACHE:
        _PROG_CACHE["nc"] = _trace_program()
    nc = _PROG_CACHE["nc"]

    cw, cr = _host_consts(np.asarray(conv_w, np.float32),
                          np.asarray(conv_b, np.float32))
    xf, xm, xfw = _slabs(feat_moving, feat_fixed)
    in_maps = [{"xfix": xf[m], "xmov": xm[m], "xfpw": xfw[m], "cw": cw, "cr": cr}
               for m in range(NCORES)]
    res = run_bass_kernel_spmd(nc, in_maps, list(range(NCORES)))
    out = np.empty((1, 3, 64, 64, 64), np.float32)
    for m in range(NCORES):
        out[0, :, 8 * m:8 * m + 8] = res.results[m]["out"].astype(np.float32).reshape(
            8, 3, 64, 64).transpose(1, 0, 2, 3)
    return out


# revision 24
# speedup vs baseline: 1.6290x; 1.0147x over previous
"""Trainium2 Bass kernel for DPBlockVFAStandard (3D local cross-attention
displacement field).

Computation (B=1, C=32, E=16, H=W=D=64):
  fixed_emb  = conv3d(feat_fixed, w, b, pad=1)                    [E,64,64,64]
  moving_emb = conv3d(edge_pad(feat_moving,1), w, b, pad=1)       [E,66,66,66]
  scores[p](h,w,d) = <fixed_emb(h,w,d), moving_emb(h+i,w+j,d+k)>/4, p=(i,j,k)
  attn = softmax_p(scores);  disp_r = sum_p attn_p * R[p,r]       [3,64,64,64]

Sharding: H split into 8 slabs of 8 rows, one per NeuronCore; halo handled
host-side by overlapping input slabs (no collectives).

Per-core device pipeline (all matmuls fp16 -> 1 cycle/row on the PE):
  - convs use a channel-split contraction: K = 16 channels x 8 planes, so one
    PSUM tile accumulates 18 fp16 matmuls (9 taps x 2 channel halves) and
    yields SIX output planes ([96, chunk]); a trailing 4-plane window covers
    the leftover planes.  PSUM evacuated directly (Act/DVE fp32->fp16 copies)
    into the final plane-stack layout [16h+e, (w,d)].
  - moving planes staged as 3 partition-shifted stacks (fp16 SBUF->SBUF DMA)
  - issue order interleaves fixed-conv halves with attention score-product
    (t = q*m) production on DVE/Pool, so the PE never starves:
      moving conv | fixed conv w-half-0 | [issue t-muls half-0]
      fixed conv w-half-1 | [issue t-muls half-1] | attn half-0 | attn half-1
  - scores: 27 elementwise fp16 muls per half + block-diag fp16 reduce
    matmuls packing 16 offsets per PSUM tile [ (a,w,h) -> 32a+8w+h ]
  - exp on ScalarE (bias -4) -> fp16; R-codebook reduction as one fp16 matmul
    per (G, chunk) with r at 32-partition stride -> aligned PSUM consumers
  - reciprocal + muls on DVE straight from PSUM; fp16 output, host upcasts
"""

import os

import numpy as np

EMBED = 16
C = 32
H = 64
NCORES = 8
ROWS = H // NCORES          # 8 output rows per core
TEMP = 4.0
EXP_BIAS = -4.0

GP_EVERY = int(os.environ.get("GP_EVERY", "4"))   # every Nth score mul -> Pool
TBUFS = int(os.environ.get("TBUFS", "20"))        # t-tile backlog depth
KREP = int(os.environ.get("KREP", "1"))           # body repetitions (timing probe)
INTERLEAVE = int(os.environ.get("INTERLEAVE", "1"))  # overlap fixed conv w/ t-muls

# cw column map
MOV_CS = 0          # moving c-split blocks: (jk*2+kap)*96, 18 x [128,96]
FIX_CS = 1728       # fixed c-split blocks
FIX_PW = 3456       # fixed pair-window blocks: jk*32, 9 x [128,32]
LRED = 3744         # 4 variants [128,32]
CWCOLS = 3872

_PROG_CACHE = {}


def _radial():
    c = np.array([-1.0, 0.0, 1.0], np.float32)
    R = np.zeros((27, 3), np.float32)
    for p in range(27):
        i, j, k = p // 9, (p // 3) % 3, p % 3
        R[p] = (c[i], c[j], c[k])
    return R


def _host_consts(conv_w, conv_b):
    """Build packed lhsT constant matrices (fp16).

    c-split block (kap, j, k) of weights wa: [128, 96] with
      row (16*pl + cc), col (16*oo + e) = wa[e, 16*kap+cc, pl-oo, j, k]
      for 0 <= pl-oo <= 2 (pl: window plane 0..7, oo: output plane 0..5)
    pair-window block (j,k): [128, 32]: cols 0:16 out-parity0 (K blocks 0..2),
      cols 16:32 parity1 (K blocks 1..3)
    LRED variant w: [128, 32], col (8w+h) sums partitions (16h..16h+16)
    cr [128, 256] fp16: blocks G of [128,128]:
      row (32a+8w+h), col (32r+h') = ind(h==h')*wr(p), p=G*16+4w+a,
      wr = R[p,r] for r<3 else 1 (r=3 -> denominator)
    """
    w = conv_w.astype(np.float32)          # [E, C, 3, 3, 3]
    wm = w / TEMP
    cw = np.zeros((128, CWCOLS), np.float32)

    def cs_block(wa, kap, j, k):
        M = np.zeros((128, 96), np.float32)
        for oo in range(6):
            for i in range(3):
                pl = oo + i
                # rows 16*pl .. +16 (cc), cols 16*oo .. +16 (e)
                M[16 * pl:16 * pl + 16, 16 * oo:16 * oo + 16] = \
                    wa[:, 16 * kap:16 * kap + 16, i, j, k].T
        return M

    for jk in range(9):
        j, k = jk // 3, jk % 3
        for kap in range(2):
            cw[:, MOV_CS + (jk * 2 + kap) * 96:MOV_CS + (jk * 2 + kap) * 96 + 96] = \
                cs_block(wm, kap, j, k)
            cw[:, FIX_CS + (jk * 2 + kap) * 96:FIX_CS + (jk * 2 + kap) * 96 + 96] = \
                cs_block(w, kap, j, k)
        for i in range(3):
            blk = w[:, :, i, j, k].T       # [C, E]
            cw[32 * i:32 * i + 32, FIX_PW + jk * 32:FIX_PW + jk * 32 + 16] = blk
            cw[32 * (i + 1):32 * (i + 1) + 32,
               FIX_PW + jk * 32 + 16:FIX_PW + jk * 32 + 32] = blk
    for wv in range(4):
        base = LRED + 32 * wv
        for h in range(8):
            cw[16 * h:16 * h + 16, base + 8 * wv + h] = 1.0

    R = _radial()
    cr = np.zeros((128, 256), np.float32)
    for G in range(2):
        npg = 16 if G == 0 else 11
        for idx in range(npg):
            p = G * 16 + idx
            a, wv = idx % 4, idx // 4
            for r in range(4):
                val = R[p, r] if r < 3 else 1.0
                for h in range(8):
                    cr[32 * a + 8 * wv + h, G * 128 + 32 * r + h] = val
    return cw.astype(np.float16), cr.astype(np.float16)


def _trace_program():
    import concourse.bacc as bacc
    import concourse.tile as tile
    import concourse.mybir as mybir
    from contextlib import ExitStack

    f32 = mybir.dt.float32
    fp16 = mybir.dt.float16
    Exp = mybir.ActivationFunctionType.Exp

    nc = bacc.Bacc("TRN2", target_bir_lowering=False, debug=False,
                   enable_asserts=True, num_devices=NCORES)
    xfix = nc.dram_tensor("xfix", [2, 10, 16, 66 * 66], fp16, kind="ExternalInput")
    xmov = nc.dram_tensor("xmov", [2, 12, 16, 68 * 68], fp16, kind="ExternalInput")
    xfpw = nc.dram_tensor("xfpw", [4, C, 66 * 66], fp16, kind="ExternalInput")
    cw_t = nc.dram_tensor("cw", [128, CWCOLS], fp16, kind="ExternalInput")
    cr_t = nc.dram_tensor("cr", [128, 256], fp16, kind="ExternalInput")
    out_t = nc.dram_tensor("out", [ROWS, 3, 64 * 64], fp16, kind="ExternalOutput")

    evac_ct = [0]

    def evac(dst, src, eng=None):
        if eng == "act" or (eng is None and evac_ct[0] % 2 == 0):
            nc.scalar.copy(dst, src)
        else:
            nc.vector.tensor_copy(dst, src)
        evac_ct[0] += 1

    with tile.TileContext(nc) as tc, \
         nc.allow_low_precision(reason="fp16 softmax weights; 2e-2 tolerance"):
      for _rep in range(KREP):
       with ExitStack() as ctx:
        cpool = ctx.enter_context(tc.tile_pool(name="consts", bufs=1))
        cwm = cpool.tile([128, 1728], fp16)
        nc.sync.dma_start(cwm[:], cw_t[:, 0:1728])
        cwf = cpool.tile([128, CWCOLS - 1728], fp16)
        crt = cpool.tile([128, 256], fp16)
        ebias = cpool.tile([128, 1], f32)
        nc.vector.memset(ebias[:], EXP_BIAS)

        mpool = ctx.enter_context(tc.tile_pool(name="stacks", bufs=1))
        mlin0 = mpool.tile([128, 66 * 66], fp16)
        mlin1 = mpool.tile([32, 66 * 66], fp16)
        mset1 = mpool.tile([128, 66 * 66], fp16)
        mset2 = mpool.tile([128, 66 * 66], fp16)
        qh = [mpool.tile([128, 2048], fp16, name=f"qh{i}") for i in range(2)]

        # ---------------- moving conv (c-split windows) ----------------
        # window W0: input planes W0..W0+7, outputs m-planes W0+oo, oo=0..5
        # DMA issue order: moving inputs first (the PE waits on them), then
        # the remaining consts and fixed-conv inputs fill the SP queue and
        # land well before their consumers need them.
        # pre-allocate fixed-conv input tiles (DMAs issued after moving's)
        xfp = ctx.enter_context(tc.tile_pool(name="xf", bufs=3))
        xf_tiles = [xfp.tile([128, 66 * 66], fp16, name=f"xft_{i}", tag="xf")
                    for i in range(3)]
        xf1 = [xf_tiles[i][:].rearrange("p (a b) -> p a b", b=66)
               for i in range(2)]
        xf2 = xf_tiles[2][:].rearrange("p (a b) -> p a b", b=66)

        cpsp = ctx.enter_context(tc.tile_pool(name="cps", bufs=2, space="PSUM"))
        with tc.tile_pool(name="xm", bufs=4) as xmp:
            xtv = {}
            for W0 in (0, 4):
                for kap in range(2):
                    xt = xmp.tile([128, 68 * 68], fp16, name=f"xm_{W0}_{kap}",
                                  tag="xm")
                    nc.sync.dma_start(
                        xt[:], xmov[kap, W0:W0 + 8, :, :]
                        .rearrange("q c n -> (q c) n"))
                    xtv[(W0, kap)] = xt[:].rearrange("p (a b) -> p a b", b=68)
            nc.sync.dma_start(cwf[:], cw_t[:, 1728:])
            nc.sync.dma_start(crt[:], cr_t[:])
            for kap in range(2):
                nc.sync.dma_start(
                    xf_tiles[kap][:],
                    xfix[kap, 0:8, :, :].rearrange("q c n -> (q c) n"))
            nc.sync.dma_start(xf_tiles[2][:],
                              xfpw[:].rearrange("q c n -> (q c) n"))
            for W0 in (0, 4):
                xts = [xtv[(W0, 0)], xtv[(W0, 1)]]
                for cidx in range(11):
                    w0 = cidx * 6
                    psb = cpsp.tile([128, 512], f32, name=f"psm_{W0}_{cidx}",
                                    tag="ps")
                    ps = psb
                    for kap in range(2):
                        for jk in range(9):
                            j, k = jk // 3, jk % 3
                            rhs = xts[kap][:, w0 + j:w0 + j + 6, k:k + 66]
                            nc.tensor.matmul(
                                ps[0:96, :396],
                                lhsT=cwm[:, (jk * 2 + kap) * 96:
                                         (jk * 2 + kap) * 96 + 96],
                                rhs=rhs,
                                start=(jk == 0 and kap == 0),
                                stop=(jk == 8 and kap == 1))
                    cs = slice(cidx * 396, (cidx + 1) * 396)
                    if W0 == 0:
                        evac(mlin0[0:96, cs], ps[0:96, :396])
                    else:
                        evac(mlin0[96:128, cs], ps[32:64, :396], eng="act")
                        evac(mlin1[0:32, cs], ps[64:96, :396], eng="act")

        # moving stacks, partition-shifted: mset_i[16h+e] = plane (h+i)
        nc.sync.dma_start(mset1[0:112, :], mlin0[16:128, :])
        nc.gpsimd.dma_start(mset2[0:96, :], mlin0[32:128, :])
        nc.sync.dma_start(mset1[112:128, :], mlin1[0:16, :])
        nc.gpsimd.dma_start(mset2[96:128, :], mlin1[0:32, :])

        # ---------------- fixed conv + attention (interleaved) ----------
        apool = ctx.enter_context(tc.tile_pool(name="tmul", bufs=TBUFS))
        epool = ctx.enter_context(tc.tile_pool(name="etile", bufs=8))
        rpool = ctx.enter_context(tc.tile_pool(name="recd", bufs=2))
        opool = ctx.enter_context(tc.tile_pool(name="outb", bufs=1))
        s4p = ctx.enter_context(tc.tile_pool(name="s4", bufs=4, space="PSUM"))
        ndp = ctx.enter_context(tc.tile_pool(name="nd", bufs=2, space="PSUM"))
        out1 = opool.tile([8, 3 * 64 * 64], fp16)
        msets = (mlin0, mset1, mset2)
        mul_ct = [0]

        def fixed_conv_half(half, eng):
            # q-planes 0..5 via c-split window [0-7]; 6,7 via pair window [6-9]
            for cidx in range(half * 4, half * 4 + 4):
                w0 = cidx * 8
                # one [128,512] psum tile: partitions 0:96 c-split (q-planes
                # 0..5), 96:128 pair-window (q-planes 6,7) -> single evac
                ps = cpsp.tile([128, 512], f32, name=f"psf_{cidx}", tag="ps")
                for jk in range(9):
                    j, k = jk // 3, jk % 3
                    for kap in range(2):
                        rhs = xf1[kap][:, w0 + j:w0 + j + 8, k:k + 64]
                        nc.tensor.matmul(
                            ps[0:96, :],
                            lhsT=cwf[:, FIX_CS - 1728 + (jk * 2 + kap) * 96:
                                     FIX_CS - 1728 + (jk * 2 + kap) * 96 + 96],
                            rhs=rhs,
                            start=(jk == 0 and kap == 0),
                            stop=(jk == 8 and kap == 1))
                for jk in range(9):
                    j, k = jk // 3, jk % 3
                    rhs = xf2[:, w0 + j:w0 + j + 8, k:k + 64]
                    nc.tensor.matmul(
                        ps[96:128, :],
                        lhsT=cwf[:, FIX_PW - 1728 + jk * 32:
                                 FIX_PW - 1728 + jk * 32 + 32],
                        rhs=rhs,
                        start=(jk == 0), stop=(jk == 8),
                        tile_position=(0, 96))
                cs = slice((cidx - half * 4) * 512, (cidx - half * 4) * 512 + 512)
                evac(qh[half][:, cs], ps[:, :], eng=eng)

        def issue_tmuls(half):
            tlist = []
            for G in (0, 1):
                npg = 16 if G == 0 else 11
                for idx in range(npg):
                    p = G * 16 + idx
                    i, j, k = p // 9, (p // 3) % 3, p % 3
                    m3 = msets[i][:].rearrange("p (a b) -> p a b", b=66)
                    msrc = m3[:, half * 32 + j:half * 32 + j + 32, k:k + 64]
                    t = apool.tile([128, 2048], fp16, name=f"t_{half}_{p}", tag="t")
                    eng = nc.gpsimd if (mul_ct[0] % GP_EVERY == GP_EVERY - 1) \
                        else nc.vector
                    eng.tensor_mul(t[:], qh[half][:], msrc)
                    mul_ct[0] += 1
                    tlist.append(t)
            return tlist

        def attn_half(half, tlist):
            fo = half * 2048
            e_tiles = {}
            ti = 0
            for G in (0, 1):
                npg = 16 if G == 0 else 11
                s4_tiles = [s4p.tile([128, 512], f32, name=f"s4_{half}_{G}_{ci}",
                                     tag="s4") for ci in range(4)]
                if G == 1:
                    # partitions 112:128 get no matmul write; zero them so
                    # exp(stale psum) can't inf out
                    for ci in range(4):
                        nc.scalar.memzero(s4_tiles[ci][96:128, :])
                for idx in range(npg):
                    t = tlist[ti]
                    ti += 1
                    a, wv = idx % 4, idx // 4
                    last_w = (npg - 1 - a) // 4
                    for ci in range(4):
                        nc.tensor.matmul(
                            s4_tiles[ci][32 * a:32 * a + 32, :],
                            lhsT=cwf[:, LRED - 1728 + 32 * wv:
                                     LRED - 1728 + 32 * wv + 32],
                            rhs=t[:, ci * 512:(ci + 1) * 512],
                            start=(wv == 0), stop=(wv == last_w),
                            tile_position=(0, 32 * a))
                for ci in range(4):
                    e = epool.tile([128, 512], fp16, name=f"e_{half}_{G}_{ci}",
                                   tag="e")
                    nc.scalar.activation(e[:], s4_tiles[ci][:], Exp, bias=ebias[:])
                    e_tiles[(G, ci)] = e
            for ci in range(4):
                nd = ndp.tile([128, 512], f32, name=f"nd_{half}_{ci}", tag="nd")
                for G in (0, 1):
                    nc.tensor.matmul(
                        nd[:, :],
                        lhsT=crt[:, G * 128:G * 128 + 128],
                        rhs=e_tiles[(G, ci)][:],
                        start=(G == 0), stop=(G == 1))
                rec = rpool.tile([8, 512], fp16, name="recd", tag="rec")
                nc.vector.reciprocal(rec[:], nd[96:104, :])
                for r in range(3):
                    nc.vector.tensor_mul(
                        out1[:, r * 4096 + fo + ci * 512:
                             r * 4096 + fo + (ci + 1) * 512],
                        nd[32 * r:32 * r + 8, :], rec[:])

        if INTERLEAVE:
            fixed_conv_half(0, "act")  # PE: fixed w-half-0; evacs Act only
            t0 = issue_tmuls(0)        # DVE/Pool: half-0 products (overlap next)
            fixed_conv_half(1, "act")  # PE: fixed w-half-1; evacs Act only
            t1 = issue_tmuls(1)        # DVE/Pool: half-1 products
            attn_half(0, t0)
            attn_half(1, t1)
        else:
            fixed_conv_half(0, None)
            fixed_conv_half(1, None)
            t0 = issue_tmuls(0)
            attn_half(0, t0)
            t1 = issue_tmuls(1)
            attn_half(1, t1)

        nc.sync.dma_start(out_t[:].rearrange("h r n -> h (r n)"), out1[:])

    nc.compile()
    return nc


def _slabs(feat_moving, feat_fixed):
    fm = np.asarray(feat_moving, np.float32)[0]   # [C, H, W, D]
    ff = np.asarray(feat_fixed, np.float32)[0]
    fixp = np.zeros((C, 66, 66, 66), np.float16)
    fixp[:, 1:65, 1:65, 1:65] = ff
    mp = np.pad(fm, ((0, 0), (1, 1), (1, 1), (1, 1)), mode="edge")
    movpp = np.zeros((C, 68, 68, 68), np.float16)
    movpp[:, 1:67, 1:67, 1:67] = mp
    xf, xm, xfw = [], [], []
    for m in range(NCORES):
        f = fixp[:, 8 * m:8 * m + 10].reshape(2, 16, 10, 66 * 66).transpose(0, 2, 1, 3)
        xf.append(np.ascontiguousarray(f))
        fw = fixp[:, 8 * m + 6:8 * m + 10].reshape(C, 4, 66 * 66).transpose(1, 0, 2)
        xfw.append(np.ascontiguousarray(fw))
        v = movpp[:, 8 * m:8 * m + 12].reshape(2, 16, 12, 68 * 68).transpose(0, 2, 1, 3)
        xm.append(np.ascontiguousarray(v))
    return xf, xm, xfw


def kernel(feat_moving, feat_fixed, conv_w, conv_b):
    from concourse.bass_utils import run_bass_kernel_spmd

    if "nc" not in _PROG_CACHE:
        _PROG_CACHE["nc"] = _trace_program()
    nc = _PROG_CACHE["nc"]

    cw, cr = _host_consts(np.asarray(conv_w, np.float32),
                          np.asarray(conv_b, np.float32))
    xf, xm, xfw = _slabs(feat_moving, feat_fixed)
    in_maps = [{"xfix": xf[m], "xmov": xm[m], "xfpw": xfw[m], "cw": cw, "cr": cr}
               for m in range(NCORES)]
    res = run_bass_kernel_spmd(nc, in_maps, list(range(NCORES)))
    out = np.empty((1, 3, 64, 64, 64), np.float32)
    for m in range(NCORES):
        out[0, :, 8 * m:8 * m + 8] = res.results[m]["out"].astype(np.float32).reshape(
            8, 3, 64, 64).transpose(1, 0, 2, 3)
    return out
